# revision 1
# baseline (speedup 1.0000x reference)
"""Causal self-attention Trainium2 kernel (B=4, T=2048, D=1024, H=16).

Sharding: 8 cores = 4 batches x 2 head-groups (8 heads each). Each core
computes its batch's qkv projection restricted to its 8 heads, causal
attention for those heads, and a partial out-projection over its 512 ctx
channels. Host sums the two partials per batch and adds b_out.

Per-core layout choices (all matmuls bf16 with fp32 PSUM accumulation):
  - xT [C, T]: channels on partitions (contraction dim for projections).
  - qkT: per head-pair p, a q-tile [128, T] (head A rows 0:64, head B rows
    64:128) and a k-tile [128, T]. Produced directly transposed by making
    W the stationary operand. The 1/sqrt(dk) scale is folded into Wq/bq.
  - scoresT[s, t] blocks [128, 512]: lhsT=kT (K=64 rows), rhs=qT. Heads A/B
    are row-packed (tile_position rows 0:64 / 64:128) and run concurrently.
    Diagonal blocks only compute the causally needed t-range.
  - causal mask: diagonal 128x128 squares get an extra K=128 identity
    matmul accumulating a {0, -30000} triangular pattern; exp() gives 0.
  - softmax: no max-subtraction (scores are within +-10 by construction),
    exp on ScalarE PSUM->SBUF bf16.
  - ctx: v stored naturally [s, d] with a ones column appended per head
    (v_ext [128, 8*65]); lhsT=v_ext (M=65) so PSUM row 64 accumulates the
    softmax denominator. Normalize = reciprocal_approx_fast + gpsimd
    partition_broadcast + DVE mul into the bf16 ctxT copy.
  - out projection: ctxT pair-tiles [128, T] are the stationary operand
    against W_outT; b_out is added on the host (once per batch).

The main loop is i-tile-outer (t blocks of 512) so qk/v projection work,
attention for all 4 pairs, and the out-projection interleave: the PE
stays dense (HAM stays at K=8/8) and ScalarE exp overlaps matmuls.
"""

import math

import numpy as np
import ml_dtypes

B, T, C = 4, 2048, 1024
H, DK = 16, 64
NCORES = 8
TS = 128  # s-tile (partition granularity)
TSL = 512  # t free-dim tile (one PSUM bank of fp32)
MASK_VAL = -30000.0
BF16 = ml_dtypes.bfloat16


def build_program(C_sz=C, T_sz=T, n_pairs=4, num_devices=1):
    import concourse.mybir as mybir
    from concourse import bacc
    from concourse.tile import TileContext

    dt = mybir.dt
    f32 = dt.float32
    bf16 = dt.bfloat16
    AF = mybir.ActivationFunctionType

    n_ct = C_sz // 128  # contraction tiles for projections
    n_qk = 2 * n_pairs  # qk o-tiles (128 channels each)
    VW = n_pairs * 2 * DK  # v channels (natural order)
    n_tt = T_sz // TS
    n_it = T_sz // TSL
    JPI = TSL // TS  # s-tiles per i-tile (4)
    OW = min(TSL, C_sz)  # output column tile width
    n_oh = C_sz // OW  # output column halves
    VEW = n_pairs * 2 * (DK + 1)  # v_ext width (65 per head)

    nc = bacc.Bacc(
        "TRN2",
        target_bir_lowering=False,
        debug=False,
        num_devices=num_devices,
    )

    xT_d = nc.dram_tensor("xT", [C_sz, T_sz], bf16, kind="ExternalInput").ap()
    wqk_d = nc.dram_tensor("wqkT", [C_sz, n_qk * 128], bf16, kind="ExternalInput").ap()
    wv_d = nc.dram_tensor("wvT", [C_sz, VW], bf16, kind="ExternalInput").ap()
    bqk_d = nc.dram_tensor("bqk", [128, n_qk], f32, kind="ExternalInput").ap()
    bv_d = nc.dram_tensor("bv", [1, VW], bf16, kind="ExternalInput").ap()
    wo_d = nc.dram_tensor("woT", [n_pairs * 128, C_sz], bf16, kind="ExternalInput").ap()
    mask_d = nc.dram_tensor("masksq", [128, TS], bf16, kind="ExternalInput").ap()
    id_d = nc.dram_tensor("ident", [128, 128], bf16, kind="ExternalInput").ap()
    out_d = nc.dram_tensor("out", [T_sz, C_sz], f32, kind="ExternalOutput").ap()

    with TileContext(nc) as tc:
        with (
            tc.tile_pool(name="const", bufs=1) as const_pool,
            tc.tile_pool(name="big", bufs=1) as big_pool,
            tc.tile_pool(name="attn", bufs=10) as attn_pool,
            tc.tile_pool(name="rinv", bufs=6) as rinv_pool,
            tc.tile_pool(name="rbc", bufs=6) as rbc_pool,
            tc.tile_pool(name="outsb", bufs=6) as outsb_pool,
            tc.tile_pool(name="sc", bufs=2, space="PSUM") as sc_ps,
            tc.tile_pool(name="mm", bufs=4, space="PSUM") as mm_ps,
        ):
            # ---- weight/activation loads (first compute inputs first) ----
            xT_sb = []
            wqk_sb = []
            wv_sb = []
            for ci in range(n_ct):
                t = big_pool.tile([128, T_sz], bf16, tag=f"xT{ci}", name=f"xT{ci}")
                nc.sync.dma_start(t[:], xT_d[ci * 128 : (ci + 1) * 128, :])
                xT_sb.append(t)
                t = big_pool.tile(
                    [128, n_qk * 128], bf16, tag=f"wqk{ci}", name=f"wqk{ci}"
                )
                nc.sync.dma_start(t[:], wqk_d[ci * 128 : (ci + 1) * 128, :])
                wqk_sb.append(t)
            bqk_sb = const_pool.tile([128, n_qk], f32, tag="bqk", name="bqk")
            nc.sync.dma_start(bqk_sb[:], bqk_d)
            for ci in range(n_ct):
                t = big_pool.tile([128, VW], bf16, tag=f"wv{ci}", name=f"wv{ci}")
                nc.sync.dma_start(t[:], wv_d[ci * 128 : (ci + 1) * 128, :])
                wv_sb.append(t)
            bv_sb = const_pool.tile([1, VW], bf16, tag="bv", name="bv")
            nc.sync.dma_start(bv_sb[:], bv_d)
            bv_bc = const_pool.tile([128, VW], bf16, tag="bv_bc", name="bv_bc")
            nc.gpsimd.partition_broadcast(bv_bc[:], bv_sb[:])
            ident_sb = const_pool.tile([128, 128], bf16, tag="ident", name="ident")
            nc.sync.dma_start(ident_sb[:], id_d)
            mask_sb = const_pool.tile([128, TS], bf16, tag="mask", name="mask")
            nc.sync.dma_start(mask_sb[:], mask_d)
            wo_sb = []
            for p in range(n_pairs):
                t = big_pool.tile([128, C_sz], bf16, tag=f"wo{p}", name=f"wo{p}")
                nc.sync.dma_start(t[:], wo_d[p * 128 : (p + 1) * 128, :])
                wo_sb.append(t)

            qkT_sb = [
                big_pool.tile([128, T_sz], bf16, tag=f"qkT{ot}", name=f"qkT{ot}")
                for ot in range(n_qk)
            ]
            vext_sb = [
                big_pool.tile([128, VEW], bf16, tag=f"vext{tt}", name=f"vext{tt}")
                for tt in range(n_tt)
            ]
            ctxT_sb = [
                big_pool.tile([128, T_sz], bf16, tag=f"ctxT{p}", name=f"ctxT{p}")
                for p in range(n_pairs)
            ]

            def qk_proj(ot, i):
                ps = mm_ps.tile([128, TSL], f32, tag="mm", name="mm")
                for ci in range(n_ct):
                    nc.tensor.matmul(
                        ps[:],
                        lhsT=wqk_sb[ci][:, ot * 128 : (ot + 1) * 128],
                        rhs=xT_sb[ci][:, i * TSL : (i + 1) * TSL],
                        start=(ci == 0),
                        stop=(ci == n_ct - 1),
                    )
                nc.scalar.activation(
                    qkT_sb[ot][:, i * TSL : (i + 1) * TSL],
                    ps[:],
                    AF.Identity,
                    bias=bqk_sb[:, ot : ot + 1],
                )

            def v_proj(tt):
                ps = mm_ps.tile([128, VW], f32, tag="mm", name="mm")
                for ci in range(n_ct):
                    nc.tensor.matmul(
                        ps[:],
                        lhsT=xT_sb[ci][:, tt * TS : (tt + 1) * TS],
                        rhs=wv_sb[ci][:],
                        start=(ci == 0),
                        stop=(ci == n_ct - 1),
                    )
                vx = vext_sb[tt]
                vx3 = vx[:].rearrange("p (h e) -> p h e", e=DK + 1)
                nc.gpsimd.memset(vx3[:, :, DK : DK + 1], 1.0)
                nc.vector.scalar_tensor_tensor(
                    vx3[:, :, 0:DK],
                    ps[:].rearrange("p (h e) -> p h e", e=DK),
                    1.0,
                    bv_bc[:].rearrange("p (h e) -> p h e", e=DK),
                    op0=mybir.AluOpType.mult,
                    op1=mybir.AluOpType.add,
                )

            def out_proj(tt, oh):
                ps = mm_ps.tile([128, OW], f32, tag="mm", name="mm")
                for p in range(n_pairs):
                    nc.tensor.matmul(
                        ps[:],
                        lhsT=ctxT_sb[p][:, tt * TS : (tt + 1) * TS],
                        rhs=wo_sb[p][:, oh * OW : (oh + 1) * OW],
                        start=(p == 0),
                        stop=(p == n_pairs - 1),
                    )
                ob = outsb_pool.tile([128, OW], f32, tag="outsb", name="outsb")
                nc.scalar.activation(ob[:], ps[:], AF.Identity)
                nc.sync.dma_start(
                    out_d[tt * TS : (tt + 1) * TS, oh * OW : (oh + 1) * OW],
                    ob[:],
                )

            def attn_pair(p, i):
                qt, kt = qkT_sb[2 * p], qkT_sb[2 * p + 1]
                nj = JPI * (i + 1)
                ctxA = mm_ps.tile([DK + 1, TSL], f32, tag="mm", name="mm")
                ctxB = mm_ps.tile([DK + 1, TSL], f32, tag="mm", name="mm")
                for j in range(nj):
                    diag = j >= JPI * i
                    pi = j - JPI * i if diag else 0
                    t0 = pi * TS  # first causally-live t column in this block
                    ps = sc_ps.tile([128, 2 * TSL], f32, tag="sc", name="sc")
                    nc.tensor.matmul(
                        ps[:, t0:TSL],
                        lhsT=kt[0:64, j * TS : (j + 1) * TS],
                        rhs=qt[0:64, i * TSL + t0 : (i + 1) * TSL],
                        start=True,
                        stop=not diag,
                        skip_group_check=True,
                    )
                    nc.tensor.matmul(
                        ps[:, TSL + t0 : 2 * TSL],
                        lhsT=kt[64:128, j * TS : (j + 1) * TS],
                        rhs=qt[64:128, i * TSL + t0 : (i + 1) * TSL],
                        start=True,
                        stop=not diag,
                        skip_group_check=True,
                    )
                    if diag:
                        nc.tensor.matmul(
                            ps[:, t0 : t0 + TS],
                            lhsT=ident_sb[:],
                            rhs=mask_sb[:],
                            start=False,
                            stop=True,
                            skip_group_check=True,
                        )
                        nc.tensor.matmul(
                            ps[:, TSL + t0 : TSL + t0 + TS],
                            lhsT=ident_sb[:],
                            rhs=mask_sb[:],
                            start=False,
                            stop=True,
                            skip_group_check=True,
                        )
                    a = attn_pool.tile([128, 2 * TSL], bf16, tag="attn", name="attn")
                    a3 = a[:].rearrange("p (c w) -> p c w", c=2)
                    ps3 = ps[:].rearrange("p (c w) -> p c w", c=2)
                    nc.scalar.activation(a3[:, :, t0:TSL], ps3[:, :, t0:TSL], AF.Exp)
                    nc.tensor.matmul(
                        ctxA[:, t0:TSL],
                        lhsT=vext_sb[j][:, (2 * p) * (DK + 1) : (2 * p + 1) * (DK + 1)],
                        rhs=a[:, t0:TSL],
                        start=(j == 0),
                        stop=(j == nj - 1),
                    )
                    nc.tensor.matmul(
                        ctxB[:, t0:TSL],
                        lhsT=vext_sb[j][
                            :, (2 * p + 1) * (DK + 1) : (2 * p + 2) * (DK + 1)
                        ],
                        rhs=a[:, TSL + t0 : 2 * TSL],
                        start=(j == 0),
                        stop=(j == nj - 1),
                    )
                isl = slice(i * TSL, (i + 1) * TSL)
                for cps, rows in ((ctxA, slice(0, 64)), (ctxB, slice(64, 128))):
                    # custom-DVE ops misread PSUM on hw: bounce rowsum via SBUF
                    rs = rinv_pool.tile([1, TSL], f32, tag="rsum", name="rsum")
                    nc.vector.tensor_copy(rs[:], cps[DK : DK + 1, :])
                    r = rinv_pool.tile([1, TSL], f32, tag="rinv", name="rinv")
                    nc.vector.reciprocal_approx_fast(r[:], rs[:])
                    rbc = rbc_pool.tile([DK, TSL], f32, tag="rbc", name="rbc")
                    nc.gpsimd.partition_broadcast(rbc[:], r[:])
                    nc.vector.tensor_mul(ctxT_sb[p][rows, isl], cps[0:DK, :], rbc[:])

            # ---- main i-outer loop ----
            # projections for i+1 are emitted between attn(i) and out(i) so
            # the PE has independent work while the last pair normalizes.
            for ot in range(n_qk):
                qk_proj(ot, 0)
            for tt in range(0, JPI):
                v_proj(tt)
            for i in range(n_it):
                # pairs 0-2 of iteration i were already emitted at the end
                # of iteration i-1 (pulled ahead so ScalarE gets exp work
                # during the projection segment).
                for p in range(0 if i == 0 else 3, n_pairs):
                    attn_pair(p, i)
                if i + 1 < n_it:
                    qk_proj(0, i + 1)
                    qk_proj(1, i + 1)
                    for tt in range(JPI * (i + 1), JPI * (i + 2)):
                        v_proj(tt)
                    attn_pair(0, i + 1)
                    qk_proj(2, i + 1)
                    qk_proj(3, i + 1)
                    attn_pair(1, i + 1)
                    qk_proj(4, i + 1)
                    qk_proj(5, i + 1)
                    attn_pair(2, i + 1)
                    qk_proj(6, i + 1)
                    qk_proj(7, i + 1)
                for tt in range(JPI * i, JPI * (i + 1)):
                    for oh in range(n_oh):
                        out_proj(tt, oh)

    nc.compile()
    return nc


def make_mask_square(ts=TS):
    """[128, ts] strict lower-triangular: cell (s, t) = MASK_VAL iff s > t."""
    s = np.arange(128)[:, None]
    t = np.arange(ts)[None, :]
    return np.where(s > t, MASK_VAL, 0.0).astype(np.float32)


def make_core_inputs(x_b, W_qkv, b_qkv, W_out, heads, C_sz=C, T_sz=T):
    """Build the per-core input map (numpy, host-side)."""
    n_pairs = len(heads) // 2
    n_qk = 2 * n_pairs
    VW = len(heads) * DK
    xT = np.ascontiguousarray(x_b.T).astype(BF16)
    wqk = np.empty((C_sz, n_qk * 128), np.float32)
    bqk = np.empty((128, n_qk), np.float32)
    wv = np.empty((C_sz, VW), np.float32)
    bv = np.empty((1, VW), np.float32)
    wo = np.empty((n_pairs * 128, C_sz), np.float32)
    for p in range(n_pairs):
        hA, hB = heads[2 * p], heads[2 * p + 1]
        # q tile (scaled by 1/sqrt(dk)=1/8), k tile
        for half, h in ((0, hA), (1, hB)):
            r0 = h * 3 * DK
            wqk[:, 2 * p * 128 + half * 64 : 2 * p * 128 + half * 64 + 64] = (
                W_qkv[r0 : r0 + DK].T / math.sqrt(DK)
            )
            bqk[half * 64 : half * 64 + 64, 2 * p] = b_qkv[r0 : r0 + DK] / math.sqrt(DK)
            wqk[:, (2 * p + 1) * 128 + half * 64 : (2 * p + 1) * 128 + half * 64 + 64] = (
                W_qkv[r0 + DK : r0 + 2 * DK].T
            )
            bqk[half * 64 : half * 64 + 64, 2 * p + 1] = b_qkv[r0 + DK : r0 + 2 * DK]
            wo[p * 128 + half * 64 : p * 128 + half * 64 + 64, :] = W_out[
                :, h * DK : (h + 1) * DK
            ].T
    for hh, h in enumerate(heads):
        r0 = h * 3 * DK + 2 * DK
        wv[:, hh * DK : (hh + 1) * DK] = W_qkv[r0 : r0 + DK].T
        bv[0, hh * DK : (hh + 1) * DK] = b_qkv[r0 : r0 + DK]
    return {
        "xT": xT,
        "wqkT": wqk.astype(BF16),
        "wvT": wv.astype(BF16),
        "bqk": bqk.astype(np.float32),
        "bv": bv.astype(BF16),
        "woT": wo.astype(BF16),
        "masksq": make_mask_square().astype(BF16),
        "ident": np.eye(128, dtype=np.float32).astype(BF16),
    }


_NC_CACHE = {}


def kernel(x, W_qkv, b_qkv, W_out, b_out, _trace=False):
    x = np.asarray(x, dtype=np.float32)
    W_qkv = np.asarray(W_qkv, dtype=np.float32)
    b_qkv = np.asarray(b_qkv, dtype=np.float32)
    W_out = np.asarray(W_out, dtype=np.float32)
    b_out = np.asarray(b_out, dtype=np.float32)

    from concourse.bass_utils import run_bass_kernel_spmd

    key = ("full", C, T, 4)
    if key not in _NC_CACHE:
        _NC_CACHE[key] = build_program(C, T, n_pairs=4, num_devices=1)
    nc = _NC_CACHE[key]

    in_maps = []
    for core in range(NCORES):
        b, hg = divmod(core, 2)
        heads = list(range(hg * 8, hg * 8 + 8))
        in_maps.append(make_core_inputs(x[b], W_qkv, b_qkv, W_out, heads))

    res = run_bass_kernel_spmd(nc, in_maps, list(range(NCORES)), trace=_trace)
    kernel._last_results = res

    out = np.broadcast_to(b_out, (B, T, C)).astype(np.float32).copy()
    for core in range(NCORES):
        b = core // 2
        out[b] += res.results[core]["out"]
    return out



# revision 4
# speedup vs baseline: 1.0655x; 1.0655x over previous
"""Causal self-attention Trainium2 kernel (B=4, T=2048, D=1024, H=16).

Sharding: 8 cores = 4 batches x 2 head-groups (8 heads each). Each core
computes its batch's qkv projection restricted to its 8 heads, causal
attention for those heads, and a partial out-projection over its 512 ctx
channels. Host sums the two partials per batch and adds b_out.

Per-core layout choices (all matmuls bf16 with fp32 PSUM accumulation):
  - xT [C, T]: channels on partitions (contraction dim for projections).
  - qkT: per head-pair p, a q-tile [128, T] (head A rows 0:64, head B rows
    64:128) and a k-tile [128, T]. Produced directly transposed by making
    W the stationary operand. The 1/sqrt(dk) scale is folded into Wq/bq.
  - scoresT[s, t] blocks [128, 512]: lhsT=kT (K=64 rows), rhs=qT. Heads A/B
    are row-packed (PE row groups 0:64 / 64:128) and run concurrently.
    Diagonal blocks only compute the causally needed t-range.
  - causal mask: after exp, the diagonal 128x128 squares are multiplied
    in-place (DVE) by a {0,1} strict-lower-triangular bf16 mask.
  - softmax: no max-subtraction (scores are within +-10 by construction),
    exp on ScalarE PSUM->SBUF bf16.
  - ctx: v stored naturally [s, d] with a ones column appended per head
    (v_ext [128, 8*65]); lhsT=v_ext (M=65) so PSUM row 64 accumulates the
    softmax denominator. Normalize = reciprocal_approx_fast + gpsimd
    partition_broadcast + DVE mul into the bf16 ctxT copy.
  - out projection: ctxT pair-tiles [128, T] are the stationary operand
    against W_outT; b_out is added on the host (once per batch).

Scheduling: the per-engine instruction streams execute strictly in
emission order, so filler matmuls (next iteration's projections and the
previous iteration's out-projection) are pumped from deques into the
exact emission points where the PE would otherwise stall on ScalarE exp
(pair starts and the steady j-loop). ScalarE runs exp only; bias adds
and PSUM evacuations run on DVE. DMAs are staged in first-use order so
compute starts ~3us in.
"""

import math
from collections import deque

import numpy as np
import ml_dtypes

B, T, C = 4, 2048, 1024
H, DK = 16, 64
NCORES = 8
TS = 128  # s-tile (partition granularity)
TSL = 512  # t free-dim tile (one PSUM bank of fp32)
BF16 = ml_dtypes.bfloat16


def build_program(C_sz=C, T_sz=T, n_pairs=4, num_devices=1):
    import concourse.mybir as mybir
    from concourse import bacc
    from concourse.tile import TileContext

    dt = mybir.dt
    f32 = dt.float32
    bf16 = dt.bfloat16
    AF = mybir.ActivationFunctionType

    n_ct = C_sz // 128  # contraction tiles for projections
    n_qk = 2 * n_pairs  # qk o-tiles (128 channels each)
    VW = n_pairs * 2 * DK  # v channels (natural order)
    n_tt = T_sz // TS
    n_it = T_sz // TSL
    JPI = TSL // TS  # s-tiles per i-tile (4)
    OW = min(TSL, C_sz)  # output column tile width
    n_oh = C_sz // OW  # output column halves
    VEW = n_pairs * 2 * (DK + 1)  # v_ext width (65 per head)

    nc = bacc.Bacc(
        "TRN2",
        target_bir_lowering=False,
        debug=False,
        num_devices=num_devices,
    )

    xT_d = nc.dram_tensor("xT", [C_sz, T_sz], bf16, kind="ExternalInput").ap()
    wqk_d = nc.dram_tensor("wqkT", [C_sz, n_qk * 128], bf16, kind="ExternalInput").ap()
    wv_d = nc.dram_tensor("wvT", [C_sz, VW], bf16, kind="ExternalInput").ap()
    bqk_d = nc.dram_tensor("bqk", [128, n_qk], f32, kind="ExternalInput").ap()
    bv_d = nc.dram_tensor("bv", [1, VW], bf16, kind="ExternalInput").ap()
    wo_d = nc.dram_tensor("woT", [n_pairs * 128, C_sz], bf16, kind="ExternalInput").ap()
    mask_d = nc.dram_tensor("mask01", [128, 2 * TS], bf16, kind="ExternalInput").ap()
    out_d = nc.dram_tensor("out", [T_sz, C_sz], bf16, kind="ExternalOutput").ap()

    with TileContext(nc) as tc:
        with (
            tc.tile_pool(name="const", bufs=1) as const_pool,
            tc.tile_pool(name="big", bufs=1) as big_pool,
            tc.tile_pool(name="attn", bufs=10) as attn_pool,
            tc.tile_pool(name="rinv", bufs=6) as rinv_pool,
            tc.tile_pool(name="rbc", bufs=6) as rbc_pool,
            tc.tile_pool(name="outsb", bufs=6) as outsb_pool,
            tc.tile_pool(name="sc", bufs=2, space="PSUM") as sc_ps,
            tc.tile_pool(name="mm", bufs=4, space="PSUM") as mm_ps,
        ):
            # ---- SBUF tiles ----
            xT_sb = []
            wqk_sb = []
            wv_sb = []
            for ci in range(n_ct):
                xT_sb.append(
                    big_pool.tile([128, T_sz], bf16, tag=f"xT{ci}", name=f"xT{ci}")
                )
                wqk_sb.append(
                    big_pool.tile(
                        [128, n_qk * 128], bf16, tag=f"wqk{ci}", name=f"wqk{ci}"
                    )
                )
                wv_sb.append(big_pool.tile([128, VW], bf16, tag=f"wv{ci}", name=f"wv{ci}"))
            bqk_sb = const_pool.tile([128, n_qk], f32, tag="bqk", name="bqk")
            bv_sb = const_pool.tile([1, VW], bf16, tag="bv", name="bv")
            bv_bc = const_pool.tile([128, VW], bf16, tag="bv_bc", name="bv_bc")
            mask_sb = const_pool.tile([128, 2 * TS], bf16, tag="mask", name="mask")
            wo_sb = [
                big_pool.tile([128, C_sz], bf16, tag=f"wo{p}", name=f"wo{p}")
                for p in range(n_pairs)
            ]

            # ---- staged DMA issue: first-use order ----
            HQK = n_pairs * 128  # half of the qk o-range (ot 0..3)
            for ci in range(n_ct):
                nc.sync.dma_start(
                    xT_sb[ci][:, 0:TSL], xT_d[ci * 128 : (ci + 1) * 128, 0:TSL]
                )
                nc.sync.dma_start(
                    wqk_sb[ci][:, 0:HQK], wqk_d[ci * 128 : (ci + 1) * 128, 0:HQK]
                )
            nc.sync.dma_start(bqk_sb[:], bqk_d)
            for ci in range(n_ct):
                nc.sync.dma_start(wv_sb[ci][:], wv_d[ci * 128 : (ci + 1) * 128, :])
            nc.sync.dma_start(bv_sb[:], bv_d)
            nc.gpsimd.partition_broadcast(bv_bc[:], bv_sb[:])
            nc.sync.dma_start(mask_sb[:], mask_d)
            for ci in range(n_ct):
                nc.sync.dma_start(
                    wqk_sb[ci][:, HQK : n_qk * 128],
                    wqk_d[ci * 128 : (ci + 1) * 128, HQK : n_qk * 128],
                )
            for ci in range(n_ct):
                nc.sync.dma_start(
                    xT_sb[ci][:, TSL : 2 * TSL],
                    xT_d[ci * 128 : (ci + 1) * 128, TSL : 2 * TSL],
                )
            for p in range(n_pairs):
                nc.sync.dma_start(wo_sb[p][:], wo_d[p * 128 : (p + 1) * 128, :])
            for ii in range(2, n_it):
                for ci in range(n_ct):
                    nc.sync.dma_start(
                        xT_sb[ci][:, ii * TSL : (ii + 1) * TSL],
                        xT_d[ci * 128 : (ci + 1) * 128, ii * TSL : (ii + 1) * TSL],
                    )

            qkT_sb = [
                big_pool.tile([128, T_sz], bf16, tag=f"qkT{ot}", name=f"qkT{ot}")
                for ot in range(n_qk)
            ]
            vext_sb = [
                big_pool.tile([128, VEW], bf16, tag=f"vext{tt}", name=f"vext{tt}")
                for tt in range(n_tt)
            ]
            ctxT_sb = [
                big_pool.tile([128, T_sz], bf16, tag=f"ctxT{p}", name=f"ctxT{p}")
                for p in range(n_pairs)
            ]

            def qk_proj(ot, i):
                ps = mm_ps.tile([128, TSL], f32, tag="mm", name="mm")
                for ci in range(n_ct):
                    nc.tensor.matmul(
                        ps[:],
                        lhsT=wqk_sb[ci][:, ot * 128 : (ot + 1) * 128],
                        rhs=xT_sb[ci][:, i * TSL : (i + 1) * TSL],
                        start=(ci == 0),
                        stop=(ci == n_ct - 1),
                    )
                nc.vector.tensor_scalar_add(
                    qkT_sb[ot][:, i * TSL : (i + 1) * TSL],
                    ps[:],
                    bqk_sb[:, ot : ot + 1],
                )

            def v_proj(tt):
                ps = mm_ps.tile([128, VW], f32, tag="mm", name="mm")
                for ci in range(n_ct):
                    nc.tensor.matmul(
                        ps[:],
                        lhsT=xT_sb[ci][:, tt * TS : (tt + 1) * TS],
                        rhs=wv_sb[ci][:],
                        start=(ci == 0),
                        stop=(ci == n_ct - 1),
                    )
                vx = vext_sb[tt]
                vx3 = vx[:].rearrange("p (h e) -> p h e", e=DK + 1)
                nc.gpsimd.memset(vx3[:, :, DK : DK + 1], 1.0)
                nc.vector.scalar_tensor_tensor(
                    vx3[:, :, 0:DK],
                    ps[:].rearrange("p (h e) -> p h e", e=DK),
                    1.0,
                    bv_bc[:].rearrange("p (h e) -> p h e", e=DK),
                    op0=mybir.AluOpType.mult,
                    op1=mybir.AluOpType.add,
                )

            def out_proj(tt, oh):
                ps = mm_ps.tile([128, OW], f32, tag="mm", name="mm")
                for p in range(n_pairs):
                    nc.tensor.matmul(
                        ps[:],
                        lhsT=ctxT_sb[p][:, tt * TS : (tt + 1) * TS],
                        rhs=wo_sb[p][:, oh * OW : (oh + 1) * OW],
                        start=(p == 0),
                        stop=(p == n_pairs - 1),
                    )
                ob = outsb_pool.tile([128, OW], bf16, tag="outsb", name="outsb")
                nc.vector.tensor_copy(ob[:], ps[:])
                nc.sync.dma_start(
                    out_d[tt * TS : (tt + 1) * TS, oh * OW : (oh + 1) * OW],
                    ob[:],
                )

            # filler queues: boundary items may carry ScalarE/DVE tails that
            # would delay the exp chain mid-pair; inner items (out_proj) only
            # touch DVE, which never gates exp for non-diagonal blocks.
            boundary_q = deque()
            inner_q = deque()

            def pump(q, n):
                for _ in range(n):
                    if q:
                        q.popleft()()

            mask3 = mask_sb[:].rearrange("p (c w) -> p c w", c=2)

            def attn_pair(p, i):
                qt, kt = qkT_sb[2 * p], qkT_sb[2 * p + 1]
                nj = JPI * (i + 1)
                ctxA = mm_ps.tile([DK + 1, TSL], f32, tag="mm", name="mm")
                ctxB = mm_ps.tile([DK + 1, TSL], f32, tag="mm", name="mm")
                for j in range(nj):
                    diag = j >= JPI * i
                    pi = j - JPI * i if diag else 0
                    t0 = pi * TS  # first causally-live t column in this block
                    ps = sc_ps.tile([128, 2 * TSL], f32, tag="sc", name="sc")
                    nc.tensor.matmul(
                        ps[:, t0:TSL],
                        lhsT=kt[0:64, j * TS : (j + 1) * TS],
                        rhs=qt[0:64, i * TSL + t0 : (i + 1) * TSL],
                        start=True,
                        stop=True,
                        skip_group_check=True,
                    )
                    nc.tensor.matmul(
                        ps[:, TSL + t0 : 2 * TSL],
                        lhsT=kt[64:128, j * TS : (j + 1) * TS],
                        rhs=qt[64:128, i * TSL + t0 : (i + 1) * TSL],
                        start=True,
                        stop=True,
                        skip_group_check=True,
                    )
                    a = attn_pool.tile([128, 2 * TSL], bf16, tag="attn", name="attn")
                    a3 = a[:].rearrange("p (c w) -> p c w", c=2)
                    ps3 = ps[:].rearrange("p (c w) -> p c w", c=2)
                    nc.scalar.activation(a3[:, :, t0:TSL], ps3[:, :, t0:TSL], AF.Exp)
                    if diag:
                        nc.vector.tensor_mul(
                            a3[:, :, t0 : t0 + TS],
                            a3[:, :, t0 : t0 + TS],
                            mask3[:, :, :],
                        )
                    else:
                        pump(inner_q, 1)
                    nc.tensor.matmul(
                        ctxA[:, t0:TSL],
                        lhsT=vext_sb[j][:, (2 * p) * (DK + 1) : (2 * p + 1) * (DK + 1)],
                        rhs=a[:, t0:TSL],
                        start=(j == 0),
                        stop=(j == nj - 1),
                    )
                    nc.tensor.matmul(
                        ctxB[:, t0:TSL],
                        lhsT=vext_sb[j][
                            :, (2 * p + 1) * (DK + 1) : (2 * p + 2) * (DK + 1)
                        ],
                        rhs=a[:, TSL + t0 : 2 * TSL],
                        start=(j == 0),
                        stop=(j == nj - 1),
                    )
                isl = slice(i * TSL, (i + 1) * TSL)
                for cps, rows in ((ctxA, slice(0, 64)), (ctxB, slice(64, 128))):
                    # custom-DVE ops misread PSUM on hw: bounce rowsum via SBUF
                    rs = rinv_pool.tile([1, TSL], f32, tag="rsum", name="rsum")
                    nc.vector.tensor_copy(rs[:], cps[DK : DK + 1, :])
                    r = rinv_pool.tile([1, TSL], f32, tag="rinv", name="rinv")
                    nc.vector.reciprocal_approx_fast(r[:], rs[:])
                    rbc = rbc_pool.tile([DK, TSL], f32, tag="rbc", name="rbc")
                    nc.gpsimd.partition_broadcast(rbc[:], r[:])
                    nc.vector.tensor_mul(ctxT_sb[p][rows, isl], cps[0:DK, :], rbc[:])

            # ---- main schedule ----
            for ot in range(n_qk // 2):
                qk_proj(ot, 0)
            for tt in range(JPI):
                v_proj(tt)
            for i in range(n_it):
                if i + 1 < n_it:
                    for ot in range(n_qk):
                        boundary_q.append(lambda ot=ot, i=i: qk_proj(ot, i + 1))
                    for tt in range(JPI * (i + 1), JPI * (i + 2)):
                        boundary_q.append(lambda tt=tt: v_proj(tt))
                if i > 0:
                    for tt in range(JPI * (i - 1), JPI * i):
                        for oh in range(n_oh):
                            inner_q.append(lambda tt=tt, oh=oh: out_proj(tt, oh))
                for p in range(n_pairs):
                    if i == 0 and p in (1, 2):
                        # qkT for pairs 2/3 must be emitted before their use
                        qk_proj(2 * p + 2, 0)
                        qk_proj(2 * p + 3, 0)
                    if i > 0 or p > 0:
                        pump(boundary_q, 4)
                    attn_pair(p, i)
                # leftover inner items drain at iteration end
                pump(inner_q, len(inner_q))
            pump(boundary_q, len(boundary_q))
            for tt in range(JPI * (n_it - 1), JPI * n_it):
                for oh in range(n_oh):
                    out_proj(tt, oh)

    nc.compile()
    return nc


def make_mask01(ts=TS):
    """[128, 2*ts] bf16 {0,1}: cell (s, t) = 0 iff s > t, two copies."""
    s = np.arange(128)[:, None]
    t = np.arange(ts)[None, :]
    m = np.where(s > t, 0.0, 1.0).astype(np.float32)
    return np.concatenate([m, m], axis=1)


def make_core_inputs(x_b, W_qkv, b_qkv, W_out, heads, C_sz=C, T_sz=T):
    """Build the per-core input map (numpy, host-side)."""
    n_pairs = len(heads) // 2
    n_qk = 2 * n_pairs
    VW = len(heads) * DK
    xT = np.ascontiguousarray(x_b.T).astype(BF16)
    wqk = np.empty((C_sz, n_qk * 128), np.float32)
    bqk = np.empty((128, n_qk), np.float32)
    wv = np.empty((C_sz, VW), np.float32)
    bv = np.empty((1, VW), np.float32)
    wo = np.empty((n_pairs * 128, C_sz), np.float32)
    for p in range(n_pairs):
        hA, hB = heads[2 * p], heads[2 * p + 1]
        # q tile (scaled by 1/sqrt(dk)=1/8), k tile
        for half, h in ((0, hA), (1, hB)):
            r0 = h * 3 * DK
            wqk[:, 2 * p * 128 + half * 64 : 2 * p * 128 + half * 64 + 64] = (
                W_qkv[r0 : r0 + DK].T / math.sqrt(DK)
            )
            bqk[half * 64 : half * 64 + 64, 2 * p] = b_qkv[r0 : r0 + DK] / math.sqrt(DK)
            wqk[:, (2 * p + 1) * 128 + half * 64 : (2 * p + 1) * 128 + half * 64 + 64] = (
                W_qkv[r0 + DK : r0 + 2 * DK].T
            )
            bqk[half * 64 : half * 64 + 64, 2 * p + 1] = b_qkv[r0 + DK : r0 + 2 * DK]
            wo[p * 128 + half * 64 : p * 128 + half * 64 + 64, :] = W_out[
                :, h * DK : (h + 1) * DK
            ].T
    for hh, h in enumerate(heads):
        r0 = h * 3 * DK + 2 * DK
        wv[:, hh * DK : (hh + 1) * DK] = W_qkv[r0 : r0 + DK].T
        bv[0, hh * DK : (hh + 1) * DK] = b_qkv[r0 : r0 + DK]
    return {
        "xT": xT,
        "wqkT": wqk.astype(BF16),
        "wvT": wv.astype(BF16),
        "bqk": bqk.astype(np.float32),
        "bv": bv.astype(BF16),
        "woT": wo.astype(BF16),
        "mask01": make_mask01().astype(BF16),
    }


_NC_CACHE = {}


def kernel(x, W_qkv, b_qkv, W_out, b_out, _trace=False):
    x = np.asarray(x, dtype=np.float32)
    W_qkv = np.asarray(W_qkv, dtype=np.float32)
    b_qkv = np.asarray(b_qkv, dtype=np.float32)
    W_out = np.asarray(W_out, dtype=np.float32)
    b_out = np.asarray(b_out, dtype=np.float32)

    from concourse.bass_utils import run_bass_kernel_spmd

    key = ("full", C, T, 4)
    if key not in _NC_CACHE:
        _NC_CACHE[key] = build_program(C, T, n_pairs=4, num_devices=1)
    nc = _NC_CACHE[key]

    in_maps = []
    for core in range(NCORES):
        b, hg = divmod(core, 2)
        heads = list(range(hg * 8, hg * 8 + 8))
        in_maps.append(make_core_inputs(x[b], W_qkv, b_qkv, W_out, heads))

    res = run_bass_kernel_spmd(nc, in_maps, list(range(NCORES)), trace=_trace)
    kernel._last_results = res

    out = np.broadcast_to(b_out, (B, T, C)).astype(np.float32).copy()
    for core in range(NCORES):
        b = core // 2
        out[b] += np.asarray(res.results[core]["out"], dtype=np.float32)
    return out


# revision 7
# speedup vs baseline: 1.0846x; 1.0179x over previous
"""Causal self-attention Trainium2 kernel (B=4, T=2048, D=1024, H=16).

Sharding: 8 cores = 4 batches x 2 head-groups (8 heads each). Each core
computes its batch's qkv projection restricted to its 8 heads, causal
attention for those heads, and a partial out-projection over its 512 ctx
channels. Host sums the two partials per batch and adds b_out.

Per-core layout choices (all matmuls bf16 with fp32 PSUM accumulation):
  - xT [C, T]: channels on partitions (contraction dim for projections).
  - qkT: per head-pair p, a q-tile [128, T] (head A rows 0:64, head B rows
    64:128) and a k-tile [128, T]. Produced directly transposed by making
    W the stationary operand. The 1/sqrt(dk) scale is folded into Wq/bq.
  - scoresT[s, t] blocks [128, 512]: lhsT=kT (K=64 rows), rhs=qT. Heads A/B
    are row-packed (PE row groups 0:64 / 64:128) and run concurrently.
    Diagonal blocks only compute the causally needed t-range.
  - causal mask: after exp, the diagonal 128x128 squares are multiplied
    in-place (DVE) by a {0,1} strict-lower-triangular bf16 mask.
  - softmax: no max-subtraction (scores are within +-10 by construction),
    exp on ScalarE PSUM->SBUF bf16.
  - ctx: v stored naturally [s, d] with a ones column appended per head
    (v_ext [128, 8*65]); lhsT=v_ext (M=65) so PSUM row 64 accumulates the
    softmax denominator. Normalize = reciprocal_approx_fast + gpsimd
    partition_broadcast + DVE mul into the bf16 ctxT copy.
  - out projection: ctxT pair-tiles [128, T] are the stationary operand
    against W_outT; b_out is added on the host (once per batch).

Scheduling: the per-engine instruction streams execute strictly in
emission order, so filler matmuls (next iteration's projections and the
previous iteration's out-projection) are pumped from deques into the
exact emission points where the PE would otherwise stall on ScalarE exp
(pair starts and the steady j-loop). ScalarE runs exp only; bias adds
and PSUM evacuations run on DVE. DMAs are staged in first-use order so
compute starts ~3us in.
"""

import math
from collections import deque

import numpy as np
import ml_dtypes

B, T, C = 4, 2048, 1024
H, DK = 16, 64
NCORES = 8
TS = 128  # s-tile (partition granularity)
TSL = 512  # t free-dim tile (one PSUM bank of fp32)
BF16 = ml_dtypes.bfloat16


def build_program(C_sz=C, T_sz=T, n_pairs=4, num_devices=1):
    import concourse.mybir as mybir
    from concourse import bacc
    from concourse.tile import TileContext

    dt = mybir.dt
    f32 = dt.float32
    bf16 = dt.bfloat16
    AF = mybir.ActivationFunctionType

    n_ct = C_sz // 128  # contraction tiles for projections
    n_qk = 2 * n_pairs  # qk o-tiles (128 channels each)
    VW = n_pairs * 2 * DK  # v channels (natural order)
    n_tt = T_sz // TS
    n_it = T_sz // TSL
    JPI = TSL // TS  # s-tiles per i-tile (4)
    OW = min(TSL, C_sz)  # output column tile width
    n_oh = C_sz // OW  # output column halves
    VEW = n_pairs * 2 * (DK + 1)  # v_ext width (65 per head)

    nc = bacc.Bacc(
        "TRN2",
        target_bir_lowering=False,
        debug=False,
        num_devices=num_devices,
    )

    xT_d = nc.dram_tensor("xT", [C_sz, T_sz], bf16, kind="ExternalInput").ap()
    wqk_d = nc.dram_tensor("wqkT", [C_sz, n_qk * 128], bf16, kind="ExternalInput").ap()
    wv_d = nc.dram_tensor("wvT", [C_sz, VW], bf16, kind="ExternalInput").ap()
    bqk_d = nc.dram_tensor("bqk", [128, n_qk], f32, kind="ExternalInput").ap()
    bv_d = nc.dram_tensor("bv", [1, VW], bf16, kind="ExternalInput").ap()
    wo_d = nc.dram_tensor("woT", [n_pairs * 128, C_sz], bf16, kind="ExternalInput").ap()
    mask_d = nc.dram_tensor("mask01", [128, 2 * TS], bf16, kind="ExternalInput").ap()
    out_d = nc.dram_tensor("out", [T_sz, C_sz], bf16, kind="ExternalOutput").ap()

    with TileContext(nc) as tc:
        with (
            tc.tile_pool(name="const", bufs=1) as const_pool,
            tc.tile_pool(name="big", bufs=1) as big_pool,
            tc.tile_pool(name="attn", bufs=10) as attn_pool,
            tc.tile_pool(name="rinv", bufs=6) as rinv_pool,
            tc.tile_pool(name="rbc", bufs=6) as rbc_pool,
            tc.tile_pool(name="outsb", bufs=6) as outsb_pool,
            tc.tile_pool(name="sc", bufs=2, space="PSUM") as sc_ps,
            tc.tile_pool(name="mm", bufs=4, space="PSUM") as mm_ps,
        ):
            # ---- SBUF tiles ----
            xT_sb = []
            wqk_sb = []
            wv_sb = []
            for ci in range(n_ct):
                xT_sb.append(
                    big_pool.tile([128, T_sz], bf16, tag=f"xT{ci}", name=f"xT{ci}")
                )
                wqk_sb.append(
                    big_pool.tile(
                        [128, n_qk * 128], bf16, tag=f"wqk{ci}", name=f"wqk{ci}"
                    )
                )
                wv_sb.append(big_pool.tile([128, VW], bf16, tag=f"wv{ci}", name=f"wv{ci}"))
            bqk_sb = const_pool.tile([128, n_qk], f32, tag="bqk", name="bqk")
            bv_sb = const_pool.tile([1, VW], bf16, tag="bv", name="bv")
            bv_bc = const_pool.tile([128, VW], bf16, tag="bv_bc", name="bv_bc")
            mask_sb = const_pool.tile([128, 2 * TS], bf16, tag="mask", name="mask")
            wo_sb = [
                big_pool.tile([128, C_sz], bf16, tag=f"wo{p}", name=f"wo{p}")
                for p in range(n_pairs)
            ]

            # ---- staged DMA issue: first-use order ----
            HQK = n_pairs * 128  # half of the qk o-range (ot 0..3)
            for ci in range(n_ct):
                nc.sync.dma_start(
                    xT_sb[ci][:, 0:TSL], xT_d[ci * 128 : (ci + 1) * 128, 0:TSL]
                )
                nc.sync.dma_start(
                    wqk_sb[ci][:, 0:HQK], wqk_d[ci * 128 : (ci + 1) * 128, 0:HQK]
                )
            nc.sync.dma_start(bqk_sb[:], bqk_d)
            for ci in range(n_ct):
                nc.sync.dma_start(wv_sb[ci][:], wv_d[ci * 128 : (ci + 1) * 128, :])
            nc.sync.dma_start(bv_sb[:], bv_d)
            nc.gpsimd.partition_broadcast(bv_bc[:], bv_sb[:])
            nc.sync.dma_start(mask_sb[:], mask_d)
            for ci in range(n_ct):
                nc.sync.dma_start(
                    wqk_sb[ci][:, HQK : n_qk * 128],
                    wqk_d[ci * 128 : (ci + 1) * 128, HQK : n_qk * 128],
                )
            for ci in range(n_ct):
                nc.sync.dma_start(
                    xT_sb[ci][:, TSL : 2 * TSL],
                    xT_d[ci * 128 : (ci + 1) * 128, TSL : 2 * TSL],
                )
            for p in range(n_pairs):
                nc.sync.dma_start(wo_sb[p][:], wo_d[p * 128 : (p + 1) * 128, :])
            for ii in range(2, n_it):
                for ci in range(n_ct):
                    nc.sync.dma_start(
                        xT_sb[ci][:, ii * TSL : (ii + 1) * TSL],
                        xT_d[ci * 128 : (ci + 1) * 128, ii * TSL : (ii + 1) * TSL],
                    )

            qkT_sb = [
                big_pool.tile([128, T_sz], bf16, tag=f"qkT{ot}", name=f"qkT{ot}")
                for ot in range(n_qk)
            ]
            vext_sb = [
                big_pool.tile([128, VEW], bf16, tag=f"vext{tt}", name=f"vext{tt}")
                for tt in range(n_tt)
            ]
            ctxT_sb = [
                big_pool.tile([128, T_sz], bf16, tag=f"ctxT{p}", name=f"ctxT{p}")
                for p in range(n_pairs)
            ]

            def qk_proj(ot, i):
                ps = mm_ps.tile([128, TSL], f32, tag="mm", name="mm")
                for ci in range(n_ct):
                    nc.tensor.matmul(
                        ps[:],
                        lhsT=wqk_sb[ci][:, ot * 128 : (ot + 1) * 128],
                        rhs=xT_sb[ci][:, i * TSL : (i + 1) * TSL],
                        start=(ci == 0),
                        stop=(ci == n_ct - 1),
                    )
                nc.vector.tensor_scalar_add(
                    qkT_sb[ot][:, i * TSL : (i + 1) * TSL],
                    ps[:],
                    bqk_sb[:, ot : ot + 1],
                )

            def v_proj(tt):
                ps = mm_ps.tile([128, VW], f32, tag="mm", name="mm")
                for ci in range(n_ct):
                    nc.tensor.matmul(
                        ps[:],
                        lhsT=xT_sb[ci][:, tt * TS : (tt + 1) * TS],
                        rhs=wv_sb[ci][:],
                        start=(ci == 0),
                        stop=(ci == n_ct - 1),
                    )
                vx = vext_sb[tt]
                vx3 = vx[:].rearrange("p (h e) -> p h e", e=DK + 1)
                nc.gpsimd.memset(vx3[:, :, DK : DK + 1], 1.0)
                nc.vector.scalar_tensor_tensor(
                    vx3[:, :, 0:DK],
                    ps[:].rearrange("p (h e) -> p h e", e=DK),
                    1.0,
                    bv_bc[:].rearrange("p (h e) -> p h e", e=DK),
                    op0=mybir.AluOpType.mult,
                    op1=mybir.AluOpType.add,
                )

            def out_proj(tt, oh):
                ps = mm_ps.tile([128, OW], f32, tag="mm", name="mm")
                for p in range(n_pairs):
                    nc.tensor.matmul(
                        ps[:],
                        lhsT=ctxT_sb[p][:, tt * TS : (tt + 1) * TS],
                        rhs=wo_sb[p][:, oh * OW : (oh + 1) * OW],
                        start=(p == 0),
                        stop=(p == n_pairs - 1),
                    )
                ob = outsb_pool.tile([128, OW], bf16, tag="outsb", name="outsb")
                nc.vector.tensor_copy(ob[:], ps[:])
                nc.sync.dma_start(
                    out_d[tt * TS : (tt + 1) * TS, oh * OW : (oh + 1) * OW],
                    ob[:],
                )

            # filler queues: must_q (next iter's projections, deadline = end
            # of this iter) is paced 3 per pair boundary; soft_q (prev iter's
            # out-projection) is rationed across pairs so every pair's
            # exp-latency bubbles get fill, with a reserve for the last
            # pair's normalize tail.
            must_q = deque()
            soft_q = deque()
            soft_allow = [0]

            def pump(q, n):
                for _ in range(n):
                    if q:
                        q.popleft()()

            def pump_soft(n=1):
                while n > 0 and soft_q and soft_allow[0] > 0:
                    soft_q.popleft()()
                    soft_allow[0] -= 1
                    n -= 1

            mask3 = mask_sb[:].rearrange("p (c w) -> p c w", c=2)

            def attn_pair(p, i):
                qt, kt = qkT_sb[2 * p], qkT_sb[2 * p + 1]
                nj = JPI * (i + 1)
                ctxA = mm_ps.tile([DK + 1, TSL], f32, tag="mm", name="mm")
                ctxB = mm_ps.tile([DK + 1, TSL], f32, tag="mm", name="mm")
                for j in range(nj):
                    diag = j >= JPI * i
                    pi = j - JPI * i if diag else 0
                    t0 = pi * TS  # first causally-live t column in this block
                    ps = sc_ps.tile([128, 2 * TSL], f32, tag="sc", name="sc")
                    nc.tensor.matmul(
                        ps[:, t0:TSL],
                        lhsT=kt[0:64, j * TS : (j + 1) * TS],
                        rhs=qt[0:64, i * TSL + t0 : (i + 1) * TSL],
                        start=True,
                        stop=True,
                        skip_group_check=True,
                    )
                    nc.tensor.matmul(
                        ps[:, TSL + t0 : 2 * TSL],
                        lhsT=kt[64:128, j * TS : (j + 1) * TS],
                        rhs=qt[64:128, i * TSL + t0 : (i + 1) * TSL],
                        start=True,
                        stop=True,
                        skip_group_check=True,
                    )
                    a = attn_pool.tile([128, 2 * TSL], bf16, tag="attn", name="attn")
                    a3 = a[:].rearrange("p (c w) -> p c w", c=2)
                    ps3 = ps[:].rearrange("p (c w) -> p c w", c=2)
                    nc.scalar.activation(a3[:, :, t0:TSL], ps3[:, :, t0:TSL], AF.Exp)
                    if diag:
                        nc.vector.tensor_mul(
                            a3[:, :, t0 : t0 + TS],
                            a3[:, :, t0 : t0 + TS],
                            mask3[:, :, :],
                        )
                    pump_soft(1)
                    nc.tensor.matmul(
                        ctxA[:, t0:TSL],
                        lhsT=vext_sb[j][:, (2 * p) * (DK + 1) : (2 * p + 1) * (DK + 1)],
                        rhs=a[:, t0:TSL],
                        start=(j == 0),
                        stop=(j == nj - 1),
                    )
                    nc.tensor.matmul(
                        ctxB[:, t0:TSL],
                        lhsT=vext_sb[j][
                            :, (2 * p + 1) * (DK + 1) : (2 * p + 2) * (DK + 1)
                        ],
                        rhs=a[:, TSL + t0 : 2 * TSL],
                        start=(j == 0),
                        stop=(j == nj - 1),
                    )
                isl = slice(i * TSL, (i + 1) * TSL)
                for cps, rows in ((ctxA, slice(0, 64)), (ctxB, slice(64, 128))):
                    # custom-DVE ops misread PSUM on hw: bounce rowsum via SBUF
                    rs = rinv_pool.tile([1, TSL], f32, tag="rsum", name="rsum")
                    nc.vector.tensor_copy(rs[:], cps[DK : DK + 1, :])
                    r = rinv_pool.tile([1, TSL], f32, tag="rinv", name="rinv")
                    nc.vector.reciprocal_approx_fast(r[:], rs[:])
                    rbc = rbc_pool.tile([DK, TSL], f32, tag="rbc", name="rbc")
                    nc.gpsimd.partition_broadcast(rbc[:], r[:])
                    nc.vector.tensor_mul(ctxT_sb[p][rows, isl], cps[0:DK, :], rbc[:])

            # ---- main schedule ----
            for ot in range(n_qk // 2):
                qk_proj(ot, 0)
            for tt in range(JPI):
                v_proj(tt)
            for i in range(n_it):
                if i + 1 < n_it:
                    for ot in range(n_qk):
                        must_q.append(lambda ot=ot, i=i: qk_proj(ot, i + 1))
                    for tt in range(JPI * (i + 1), JPI * (i + 2)):
                        must_q.append(lambda tt=tt: v_proj(tt))
                if i > 0:
                    for tt in range(JPI * (i - 1), JPI * i):
                        for oh in range(n_oh):
                            soft_q.append(lambda tt=tt, oh=oh: out_proj(tt, oh))
                last_it = i == n_it - 1
                for p in range(n_pairs):
                    if i == 0 and p in (1, 2):
                        # qkT for pairs 2/3 must be emitted before their use
                        qk_proj(2 * p + 2, 0)
                        qk_proj(2 * p + 3, 0)
                    if i == 0:
                        if p > 0:
                            pump(must_q, 4)
                    else:
                        pump(must_q, 3)
                    denom = (n_pairs + 1 - p) if last_it else (n_pairs - p)
                    soft_allow[0] = -(-len(soft_q) // max(denom, 1))
                    attn_pair(p, i)
                # any qk/v leftovers must land before the next iteration
                pump(must_q, len(must_q))
                if not last_it:
                    soft_allow[0] = len(soft_q)
                    pump_soft(len(soft_q))
            # iter-3 reserve: fill the last pair's normalize latency
            soft_allow[0] = len(soft_q)
            pump_soft(len(soft_q))
            for tt in range(JPI * (n_it - 1), JPI * n_it):
                for oh in range(n_oh):
                    out_proj(tt, oh)

    nc.compile()
    return nc


def make_mask01(ts=TS):
    """[128, 2*ts] bf16 {0,1}: cell (s, t) = 0 iff s > t, two copies."""
    s = np.arange(128)[:, None]
    t = np.arange(ts)[None, :]
    m = np.where(s > t, 0.0, 1.0).astype(np.float32)
    return np.concatenate([m, m], axis=1)


def make_core_inputs(x_b, W_qkv, b_qkv, W_out, heads, C_sz=C, T_sz=T):
    """Build the per-core input map (numpy, host-side)."""
    n_pairs = len(heads) // 2
    n_qk = 2 * n_pairs
    VW = len(heads) * DK
    xT = np.ascontiguousarray(x_b.T).astype(BF16)
    wqk = np.empty((C_sz, n_qk * 128), np.float32)
    bqk = np.empty((128, n_qk), np.float32)
    wv = np.empty((C_sz, VW), np.float32)
    bv = np.empty((1, VW), np.float32)
    wo = np.empty((n_pairs * 128, C_sz), np.float32)
    for p in range(n_pairs):
        hA, hB = heads[2 * p], heads[2 * p + 1]
        # q tile (scaled by 1/sqrt(dk)=1/8), k tile
        for half, h in ((0, hA), (1, hB)):
            r0 = h * 3 * DK
            wqk[:, 2 * p * 128 + half * 64 : 2 * p * 128 + half * 64 + 64] = (
                W_qkv[r0 : r0 + DK].T / math.sqrt(DK)
            )
            bqk[half * 64 : half * 64 + 64, 2 * p] = b_qkv[r0 : r0 + DK] / math.sqrt(DK)
            wqk[:, (2 * p + 1) * 128 + half * 64 : (2 * p + 1) * 128 + half * 64 + 64] = (
                W_qkv[r0 + DK : r0 + 2 * DK].T
            )
            bqk[half * 64 : half * 64 + 64, 2 * p + 1] = b_qkv[r0 + DK : r0 + 2 * DK]
            wo[p * 128 + half * 64 : p * 128 + half * 64 + 64, :] = W_out[
                :, h * DK : (h + 1) * DK
            ].T
    for hh, h in enumerate(heads):
        r0 = h * 3 * DK + 2 * DK
        wv[:, hh * DK : (hh + 1) * DK] = W_qkv[r0 : r0 + DK].T
        bv[0, hh * DK : (hh + 1) * DK] = b_qkv[r0 : r0 + DK]
    return {
        "xT": xT,
        "wqkT": wqk.astype(BF16),
        "wvT": wv.astype(BF16),
        "bqk": bqk.astype(np.float32),
        "bv": bv.astype(BF16),
        "woT": wo.astype(BF16),
        "mask01": make_mask01().astype(BF16),
    }


_NC_CACHE = {}


def kernel(x, W_qkv, b_qkv, W_out, b_out, _trace=False):
    x = np.asarray(x, dtype=np.float32)
    W_qkv = np.asarray(W_qkv, dtype=np.float32)
    b_qkv = np.asarray(b_qkv, dtype=np.float32)
    W_out = np.asarray(W_out, dtype=np.float32)
    b_out = np.asarray(b_out, dtype=np.float32)

    from concourse.bass_utils import run_bass_kernel_spmd

    key = ("full", C, T, 4)
    if key not in _NC_CACHE:
        _NC_CACHE[key] = build_program(C, T, n_pairs=4, num_devices=1)
    nc = _NC_CACHE[key]

    in_maps = []
    for core in range(NCORES):
        b, hg = divmod(core, 2)
        heads = list(range(hg * 8, hg * 8 + 8))
        in_maps.append(make_core_inputs(x[b], W_qkv, b_qkv, W_out, heads))

    res = run_bass_kernel_spmd(nc, in_maps, list(range(NCORES)), trace=_trace)
    kernel._last_results = res

    out = np.broadcast_to(b_out, (B, T, C)).astype(np.float32).copy()
    for core in range(NCORES):
        b = core // 2
        out[b] += np.asarray(res.results[core]["out"], dtype=np.float32)
    return out


# revision 23
# speedup vs baseline: 1.0909x; 1.0058x over previous
"""Causal self-attention Trainium2 kernel (B=4, T=2048, D=1024, H=16).

Sharding: 8 cores = 4 batches x 2 head-groups (8 heads each). Each core
computes its batch's qkv projection restricted to its 8 heads, causal
attention for those heads, and a partial out-projection over its 512 ctx
channels. Host sums the two partials per batch and adds b_out.

Per-core layout choices (all matmuls bf16 with fp32 PSUM accumulation):
  - xT [C, T]: channels on partitions (contraction dim for projections).
  - qkT: per head-pair p, a q-tile [128, T] (head A rows 0:64, head B rows
    64:128) and a k-tile [128, T]. Produced directly transposed by making
    W the stationary operand. The 1/sqrt(dk) scale is folded into Wq/bq.
  - scoresT[s, t] blocks [128, 512]: lhsT=kT (K=64 rows), rhs=qT. Heads A/B
    are row-packed (PE row groups 0:64 / 64:128) and run concurrently.
    Diagonal blocks only compute the causally needed t-range.
  - causal mask: after exp, the diagonal 128x128 squares are multiplied
    in-place (DVE) by a {0,1} strict-lower-triangular bf16 mask.
  - softmax: no max-subtraction (scores are within +-10 by construction),
    exp on ScalarE PSUM->SBUF bf16.
  - ctx: v stored naturally [s, d] with a ones column appended per head
    (v_ext [128, 8*65]); lhsT=v_ext (M=65) so PSUM row 64 accumulates the
    softmax denominator. Normalize = reciprocal_approx_fast + gpsimd
    partition_broadcast + DVE mul into the bf16 ctxT copy.
  - out projection: ctxT pair-tiles [128, T] are the stationary operand
    against W_outT; b_out is added on the host (once per batch).

Scheduling: the per-engine instruction streams execute strictly in
emission order, so filler matmuls (next iteration's projections and the
previous iteration's out-projection) are pumped from deques into the
exact emission points where the PE would otherwise stall on ScalarE exp
(pair starts and the steady j-loop). ScalarE runs exp only; bias adds
and PSUM evacuations run on DVE. DMAs are staged in first-use order so
compute starts ~3us in.
"""

import math
from collections import deque

import numpy as np
import ml_dtypes

B, T, C = 4, 2048, 1024
H, DK = 16, 64
NCORES = 8
TS = 128  # s-tile (partition granularity)
TSL = 512  # t free-dim tile (one PSUM bank of fp32)
BF16 = ml_dtypes.bfloat16


def build_program(C_sz=C, T_sz=T, n_pairs=4, num_devices=1):
    import concourse.mybir as mybir
    from concourse import bacc
    from concourse.tile import TileContext

    dt = mybir.dt
    f32 = dt.float32
    bf16 = dt.bfloat16
    AF = mybir.ActivationFunctionType

    n_ct = C_sz // 128  # contraction tiles for projections
    n_qk = 2 * n_pairs  # qk o-tiles (128 channels each)
    VW = n_pairs * 2 * DK  # v channels (natural order)
    n_tt = T_sz // TS
    n_it = T_sz // TSL
    JPI = TSL // TS  # s-tiles per i-tile (4)
    OW = min(TSL, C_sz)  # output column tile width
    n_oh = C_sz // OW  # output column halves
    VEW = n_pairs * 2 * (DK + 1)  # v_ext width (65 per head)

    nc = bacc.Bacc(
        "TRN2",
        target_bir_lowering=False,
        debug=False,
        num_devices=num_devices,
    )

    xT_d = nc.dram_tensor("xT", [C_sz, T_sz], bf16, kind="ExternalInput").ap()
    wqk_d = nc.dram_tensor("wqkT", [C_sz, n_qk * 128], bf16, kind="ExternalInput").ap()
    wv_d = nc.dram_tensor("wvT", [C_sz, VW], bf16, kind="ExternalInput").ap()
    bqk_d = nc.dram_tensor("bqk", [128, n_qk], f32, kind="ExternalInput").ap()
    bv_d = nc.dram_tensor("bv", [1, VW], bf16, kind="ExternalInput").ap()
    wo_d = nc.dram_tensor("woT", [n_pairs * 128, C_sz], bf16, kind="ExternalInput").ap()
    mask_d = nc.dram_tensor("mask01", [128, 2 * TS], bf16, kind="ExternalInput").ap()
    out_d = nc.dram_tensor("out", [T_sz, C_sz], bf16, kind="ExternalOutput").ap()

    with TileContext(nc) as tc:
        with (
            tc.tile_pool(name="const", bufs=1) as const_pool,
            tc.tile_pool(name="big", bufs=1) as big_pool,
            tc.tile_pool(name="attn", bufs=10) as attn_pool,
            tc.tile_pool(name="rinv", bufs=6) as rinv_pool,
            tc.tile_pool(name="rbc", bufs=6) as rbc_pool,
            tc.tile_pool(name="outsb", bufs=6) as outsb_pool,
            tc.tile_pool(name="sc", bufs=2, space="PSUM") as sc_ps,
            tc.tile_pool(name="mm", bufs=4, space="PSUM") as mm_ps,
        ):
            # ---- SBUF tiles ----
            xT_sb = []
            wqk_sb = []
            wv_sb = []
            for ci in range(n_ct):
                xT_sb.append(
                    big_pool.tile([128, T_sz], bf16, tag=f"xT{ci}", name=f"xT{ci}")
                )
                wqk_sb.append(
                    big_pool.tile(
                        [128, n_qk * 128], bf16, tag=f"wqk{ci}", name=f"wqk{ci}"
                    )
                )
                wv_sb.append(big_pool.tile([128, VW], bf16, tag=f"wv{ci}", name=f"wv{ci}"))
            bqk_sb = const_pool.tile([128, n_qk], f32, tag="bqk", name="bqk")
            bv_sb = const_pool.tile([1, VW], bf16, tag="bv", name="bv")
            bv_bc = const_pool.tile([128, VW], bf16, tag="bv_bc", name="bv_bc")
            mask_sb = const_pool.tile([128, 2 * TS], bf16, tag="mask", name="mask")
            wo_sb = [
                big_pool.tile([128, C_sz], bf16, tag=f"wo{p}", name=f"wo{p}")
                for p in range(n_pairs)
            ]

            # HAM warmup: the PE clock-gate defaults to 1.2 GHz and reaches
            # 2.4 GHz only after ~3.4us of sustained matmul activity. The
            # first ~7us of the kernel are DMA-queue init with an idle PE;
            # dummy matmuls on scratch data warm the clock gate so the
            # DMA-paced ramp and everything after runs at full rate.
            # (Emitted before any other gpsimd work so the memset runs at t~0.)
            scratch_sb = const_pool.tile([128, TSL], bf16, tag="scr", name="scr")
            nc.gpsimd.memset(scratch_sb[:], 0.0)
            warm_ps = mm_ps.tile([128, TSL], f32, tag="mm", name="mm")
            for _ in range(36):
                nc.tensor.matmul(
                    warm_ps[:],
                    lhsT=scratch_sb[:, 0:128],
                    rhs=scratch_sb[:],
                    start=True,
                    stop=True,
                    skip_group_check=True,
                )

            # ---- staged DMA issue: first-use order ----
            HQK = n_pairs * 128  # half of the qk o-range (ot 0..3)
            for ci in range(n_ct):
                nc.sync.dma_start(
                    xT_sb[ci][:, 0:TSL], xT_d[ci * 128 : (ci + 1) * 128, 0:TSL]
                )
                nc.sync.dma_start(
                    wqk_sb[ci][:, 0:HQK], wqk_d[ci * 128 : (ci + 1) * 128, 0:HQK]
                )
            nc.sync.dma_start(bqk_sb[:], bqk_d)
            for ci in range(n_ct):
                nc.sync.dma_start(wv_sb[ci][:], wv_d[ci * 128 : (ci + 1) * 128, :])
            nc.sync.dma_start(bv_sb[:], bv_d)
            nc.gpsimd.partition_broadcast(bv_bc[:], bv_sb[:])
            nc.sync.dma_start(mask_sb[:], mask_d)
            for ci in range(n_ct):
                nc.sync.dma_start(
                    wqk_sb[ci][:, HQK : n_qk * 128],
                    wqk_d[ci * 128 : (ci + 1) * 128, HQK : n_qk * 128],
                )
            for ci in range(n_ct):
                nc.sync.dma_start(
                    xT_sb[ci][:, TSL : 2 * TSL],
                    xT_d[ci * 128 : (ci + 1) * 128, TSL : 2 * TSL],
                )
            for p in range(n_pairs):
                nc.sync.dma_start(wo_sb[p][:], wo_d[p * 128 : (p + 1) * 128, :])
            for ii in range(2, n_it):
                for ci in range(n_ct):
                    nc.sync.dma_start(
                        xT_sb[ci][:, ii * TSL : (ii + 1) * TSL],
                        xT_d[ci * 128 : (ci + 1) * 128, ii * TSL : (ii + 1) * TSL],
                    )

            qkT_sb = [
                big_pool.tile([128, T_sz], bf16, tag=f"qkT{ot}", name=f"qkT{ot}")
                for ot in range(n_qk)
            ]
            vext_sb = [
                big_pool.tile([128, VEW], bf16, tag=f"vext{tt}", name=f"vext{tt}")
                for tt in range(n_tt)
            ]
            ctxT_sb = [
                big_pool.tile([128, T_sz], bf16, tag=f"ctxT{p}", name=f"ctxT{p}")
                for p in range(n_pairs)
            ]

            def qk_proj(ot, i):
                ps = mm_ps.tile([128, TSL], f32, tag="mm", name="mm")
                for ci in range(n_ct):
                    nc.tensor.matmul(
                        ps[:],
                        lhsT=wqk_sb[ci][:, ot * 128 : (ot + 1) * 128],
                        rhs=xT_sb[ci][:, i * TSL : (i + 1) * TSL],
                        start=(ci == 0),
                        stop=(ci == n_ct - 1),
                    )
                nc.vector.tensor_scalar_add(
                    qkT_sb[ot][:, i * TSL : (i + 1) * TSL],
                    ps[:],
                    bqk_sb[:, ot : ot + 1],
                )

            def v_proj(tt):
                ps = mm_ps.tile([128, VW], f32, tag="mm", name="mm")
                for ci in range(n_ct):
                    nc.tensor.matmul(
                        ps[:],
                        lhsT=xT_sb[ci][:, tt * TS : (tt + 1) * TS],
                        rhs=wv_sb[ci][:],
                        start=(ci == 0),
                        stop=(ci == n_ct - 1),
                    )
                vx = vext_sb[tt]
                vx3 = vx[:].rearrange("p (h e) -> p h e", e=DK + 1)
                nc.gpsimd.memset(vx3[:, :, DK : DK + 1], 1.0)
                nc.vector.scalar_tensor_tensor(
                    vx3[:, :, 0:DK],
                    ps[:].rearrange("p (h e) -> p h e", e=DK),
                    1.0,
                    bv_bc[:].rearrange("p (h e) -> p h e", e=DK),
                    op0=mybir.AluOpType.mult,
                    op1=mybir.AluOpType.add,
                )

            def out_proj(tt, oh):
                ps = mm_ps.tile([128, OW], f32, tag="mm", name="mm")
                for p in range(n_pairs):
                    nc.tensor.matmul(
                        ps[:],
                        lhsT=ctxT_sb[p][:, tt * TS : (tt + 1) * TS],
                        rhs=wo_sb[p][:, oh * OW : (oh + 1) * OW],
                        start=(p == 0),
                        stop=(p == n_pairs - 1),
                    )
                ob = outsb_pool.tile([128, OW], bf16, tag="outsb", name="outsb")
                nc.vector.tensor_copy(ob[:], ps[:])
                nc.sync.dma_start(
                    out_d[tt * TS : (tt + 1) * TS, oh * OW : (oh + 1) * OW],
                    ob[:],
                )

            # filler queues: must_q (next iter's projections, deadline = end
            # of this iter) is paced 3 per pair boundary; soft_q (prev iter's
            # out-projection) is rationed across pairs so every pair's
            # exp-latency bubbles get fill, with a reserve for the last
            # pair's normalize tail.
            must_q = deque()
            soft_q = deque()
            soft_allow = [0]

            def pump(q, n):
                for _ in range(n):
                    if q:
                        q.popleft()()

            def pump_soft(n=1):
                while n > 0 and soft_q and soft_allow[0] > 0:
                    soft_q.popleft()()
                    soft_allow[0] -= 1
                    n -= 1

            mask3 = mask_sb[:].rearrange("p (c w) -> p c w", c=2)

            def score_block(p, i, j):
                """Scores + exp (+ causal mask) for one s-block; returns the
                bf16 attn tile and its first live t column."""
                qt, kt = qkT_sb[2 * p], qkT_sb[2 * p + 1]
                diag = j >= JPI * i
                pi = j - JPI * i if diag else 0
                t0 = pi * TS  # first causally-live t column in this block
                ps = sc_ps.tile([128, 2 * TSL], f32, tag="sc", name="sc")
                nc.tensor.matmul(
                    ps[:, t0:TSL],
                    lhsT=kt[0:64, j * TS : (j + 1) * TS],
                    rhs=qt[0:64, i * TSL + t0 : (i + 1) * TSL],
                    start=True,
                    stop=True,
                    skip_group_check=True,
                )
                nc.tensor.matmul(
                    ps[:, TSL + t0 : 2 * TSL],
                    lhsT=kt[64:128, j * TS : (j + 1) * TS],
                    rhs=qt[64:128, i * TSL + t0 : (i + 1) * TSL],
                    start=True,
                    stop=True,
                    skip_group_check=True,
                )
                a = attn_pool.tile([128, 2 * TSL], bf16, tag="attn", name="attn")
                a3 = a[:].rearrange("p (c w) -> p c w", c=2)
                ps3 = ps[:].rearrange("p (c w) -> p c w", c=2)
                nc.scalar.activation(a3[:, :, t0:TSL], ps3[:, :, t0:TSL], AF.Exp)
                if diag:
                    nc.vector.tensor_mul(
                        a3[:, :, t0 : t0 + TS],
                        a3[:, :, t0 : t0 + TS],
                        mask3[:, :, :],
                    )
                return a, t0

            # score blocks pre-emitted ahead of their pair's body (the last
            # pair's tail overlaps the next pair's first exp latencies)
            heads = {}

            def emit_head(p, i, nh=2):
                heads[(p, i)] = [score_block(p, i, j) for j in range(min(nh, JPI * (i + 1)))]

            def attn_pair(p, i, splice=None):
                nj = JPI * (i + 1)
                pre = heads.pop((p, i), [])
                ctxA = mm_ps.tile([DK + 1, TSL], f32, tag="mm", name="mm")
                ctxB = mm_ps.tile([DK + 1, TSL], f32, tag="mm", name="mm")
                blocks = list(pre)
                for j in range(nj):
                    # keep the score pipeline one block ahead of ctx
                    while len(blocks) <= min(j + 1, nj - 1):
                        blocks.append(score_block(p, i, len(blocks)))
                    if j == nj - 1 and splice is not None:
                        splice()  # next pair's head: exps overlap our tail
                    a, t0 = blocks[j]
                    pump_soft(1)
                    nc.tensor.matmul(
                        ctxA[:, t0:TSL],
                        lhsT=vext_sb[j][:, (2 * p) * (DK + 1) : (2 * p + 1) * (DK + 1)],
                        rhs=a[:, t0:TSL],
                        start=(j == 0),
                        stop=(j == nj - 1),
                    )
                    nc.tensor.matmul(
                        ctxB[:, t0:TSL],
                        lhsT=vext_sb[j][
                            :, (2 * p + 1) * (DK + 1) : (2 * p + 2) * (DK + 1)
                        ],
                        rhs=a[:, TSL + t0 : 2 * TSL],
                        start=(j == 0),
                        stop=(j == nj - 1),
                    )
                isl = slice(i * TSL, (i + 1) * TSL)
                for cps, rows in ((ctxA, slice(0, 64)), (ctxB, slice(64, 128))):
                    # custom-DVE ops misread PSUM on hw: bounce rowsum via SBUF
                    rs = rinv_pool.tile([1, TSL], f32, tag="rsum", name="rsum")
                    nc.vector.tensor_copy(rs[:], cps[DK : DK + 1, :])
                    r = rinv_pool.tile([1, TSL], f32, tag="rinv", name="rinv")
                    nc.vector.reciprocal_approx_fast(r[:], rs[:])
                    rbc = rbc_pool.tile([DK, TSL], f32, tag="rbc", name="rbc")
                    nc.gpsimd.partition_broadcast(rbc[:], r[:])
                    nc.vector.tensor_mul(ctxT_sb[p][rows, isl], cps[0:DK, :], rbc[:])

            # ---- main schedule ----
            for ot in range(n_qk // 2):
                qk_proj(ot, 0)
            for tt in range(JPI):
                v_proj(tt)
            emit_head(0, 0)
            for i in range(n_it):
                if i + 1 < n_it:
                    for ot in range(n_qk):
                        must_q.append(lambda ot=ot, i=i: qk_proj(ot, i + 1))
                    for tt in range(JPI * (i + 1), JPI * (i + 2)):
                        must_q.append(lambda tt=tt: v_proj(tt))
                if i > 0:
                    for tt in range(JPI * (i - 1), JPI * i):
                        for oh in range(n_oh):
                            soft_q.append(lambda tt=tt, oh=oh: out_proj(tt, oh))
                last_it = i == n_it - 1
                for p in range(n_pairs):
                    if i == 0 and p in (1, 2):
                        # qkT for pairs 2/3 must be emitted before their use
                        # (pair p+1's head is spliced into pair p's body)
                        qk_proj(2 * p + 2, 0)
                        qk_proj(2 * p + 3, 0)
                    if i == 0:
                        if p > 0:
                            pump(must_q, 4)
                    else:
                        pump(must_q, 3)
                    denom = (n_pairs + 1 - p) if last_it else (n_pairs - p)
                    soft_allow[0] = -(-len(soft_q) // max(denom, 1))
                    if p + 1 < n_pairs:
                        nxt = (p + 1, i)
                    elif i + 1 < n_it:
                        nxt = (0, i + 1)
                    else:
                        nxt = None
                    attn_pair(
                        p,
                        i,
                        splice=(lambda nxt=nxt: emit_head(*nxt)) if nxt else None,
                    )
                # any qk/v leftovers must land before the next iteration
                pump(must_q, len(must_q))
                if not last_it:
                    soft_allow[0] = len(soft_q)
                    pump_soft(len(soft_q))
            # iter-3 reserve: fill the last pair's normalize latency
            soft_allow[0] = len(soft_q)
            pump_soft(len(soft_q))
            for tt in range(JPI * (n_it - 1), JPI * n_it):
                for oh in range(n_oh):
                    out_proj(tt, oh)

    nc.compile()
    return nc


def make_mask01(ts=TS):
    """[128, 2*ts] bf16 {0,1}: cell (s, t) = 0 iff s > t, two copies."""
    s = np.arange(128)[:, None]
    t = np.arange(ts)[None, :]
    m = np.where(s > t, 0.0, 1.0).astype(np.float32)
    return np.concatenate([m, m], axis=1)


def make_core_inputs(x_b, W_qkv, b_qkv, W_out, heads, C_sz=C, T_sz=T):
    """Build the per-core input map (numpy, host-side)."""
    n_pairs = len(heads) // 2
    n_qk = 2 * n_pairs
    VW = len(heads) * DK
    xT = np.ascontiguousarray(x_b.T).astype(BF16)
    wqk = np.empty((C_sz, n_qk * 128), np.float32)
    bqk = np.empty((128, n_qk), np.float32)
    wv = np.empty((C_sz, VW), np.float32)
    bv = np.empty((1, VW), np.float32)
    wo = np.empty((n_pairs * 128, C_sz), np.float32)
    for p in range(n_pairs):
        hA, hB = heads[2 * p], heads[2 * p + 1]
        # q tile (scaled by 1/sqrt(dk)=1/8), k tile
        for half, h in ((0, hA), (1, hB)):
            r0 = h * 3 * DK
            wqk[:, 2 * p * 128 + half * 64 : 2 * p * 128 + half * 64 + 64] = (
                W_qkv[r0 : r0 + DK].T / math.sqrt(DK)
            )
            bqk[half * 64 : half * 64 + 64, 2 * p] = b_qkv[r0 : r0 + DK] / math.sqrt(DK)
            wqk[:, (2 * p + 1) * 128 + half * 64 : (2 * p + 1) * 128 + half * 64 + 64] = (
                W_qkv[r0 + DK : r0 + 2 * DK].T
            )
            bqk[half * 64 : half * 64 + 64, 2 * p + 1] = b_qkv[r0 + DK : r0 + 2 * DK]
            wo[p * 128 + half * 64 : p * 128 + half * 64 + 64, :] = W_out[
                :, h * DK : (h + 1) * DK
            ].T
    for hh, h in enumerate(heads):
        r0 = h * 3 * DK + 2 * DK
        wv[:, hh * DK : (hh + 1) * DK] = W_qkv[r0 : r0 + DK].T
        bv[0, hh * DK : (hh + 1) * DK] = b_qkv[r0 : r0 + DK]
    return {
        "xT": xT,
        "wqkT": wqk.astype(BF16),
        "wvT": wv.astype(BF16),
        "bqk": bqk.astype(np.float32),
        "bv": bv.astype(BF16),
        "woT": wo.astype(BF16),
        "mask01": make_mask01().astype(BF16),
    }


_NC_CACHE = {}


def kernel(x, W_qkv, b_qkv, W_out, b_out, _trace=False):
    x = np.asarray(x, dtype=np.float32)
    W_qkv = np.asarray(W_qkv, dtype=np.float32)
    b_qkv = np.asarray(b_qkv, dtype=np.float32)
    W_out = np.asarray(W_out, dtype=np.float32)
    b_out = np.asarray(b_out, dtype=np.float32)

    from concourse.bass_utils import run_bass_kernel_spmd

    key = ("full", C, T, 4)
    if key not in _NC_CACHE:
        _NC_CACHE[key] = build_program(C, T, n_pairs=4, num_devices=1)
    nc = _NC_CACHE[key]

    in_maps = []
    for core in range(NCORES):
        b, hg = divmod(core, 2)
        heads = list(range(hg * 8, hg * 8 + 8))
        in_maps.append(make_core_inputs(x[b], W_qkv, b_qkv, W_out, heads))

    res = run_bass_kernel_spmd(nc, in_maps, list(range(NCORES)), trace=_trace)
    kernel._last_results = res

    out = np.broadcast_to(b_out, (B, T, C)).astype(np.float32).copy()
    for core in range(NCORES):
        b = core // 2
        out[b] += np.asarray(res.results[core]["out"], dtype=np.float32)
    return out


# revision 25
# speedup vs baseline: 1.1007x; 1.0089x over previous
"""Causal self-attention Trainium2 kernel (B=4, T=2048, D=1024, H=16).

Sharding: 8 cores = 4 batches x 2 head-groups (8 heads each). Each core
computes its batch's qkv projection restricted to its 8 heads, causal
attention for those heads, and a partial out-projection over its 512 ctx
channels. Host sums the two partials per batch and adds b_out.

Per-core layout choices (all matmuls bf16 with fp32 PSUM accumulation):
  - xT [C, T]: channels on partitions (contraction dim for projections).
  - qkT: per head-pair p, a q-tile [128, T] (head A rows 0:64, head B rows
    64:128) and a k-tile [128, T]. Produced directly transposed by making
    W the stationary operand. The 1/sqrt(dk) scale is folded into Wq/bq.
  - scoresT[s, t] blocks [128, 512]: lhsT=kT (K=64 rows), rhs=qT. Heads A/B
    are row-packed (PE row groups 0:64 / 64:128) and run concurrently.
    Diagonal blocks only compute the causally needed t-range.
  - causal mask: after exp, the diagonal 128x128 squares are multiplied
    in-place (DVE) by a {0,1} strict-lower-triangular bf16 mask.
  - softmax: no max-subtraction (scores are within +-10 by construction),
    exp on ScalarE PSUM->SBUF bf16.
  - ctx: v stored naturally [s, d] with a ones column appended per head
    (v_ext [128, 8*65]); lhsT=v_ext (M=65) so PSUM row 64 accumulates the
    softmax denominator. Normalize = reciprocal_approx_fast + gpsimd
    partition_broadcast + DVE mul into the bf16 ctxT copy.
  - out projection: ctxT pair-tiles [128, T] are the stationary operand
    against W_outT; b_out is added on the host (once per batch).

Scheduling: the per-engine instruction streams execute strictly in
emission order, so filler matmuls (next iteration's projections and the
previous iteration's out-projection) are pumped from deques into the
exact emission points where the PE would otherwise stall on ScalarE exp
(pair starts and the steady j-loop). ScalarE runs exp only; bias adds
and PSUM evacuations run on DVE. DMAs are staged in first-use order so
compute starts ~3us in.
"""

import math
from collections import deque

import numpy as np
import ml_dtypes

B, T, C = 4, 2048, 1024
H, DK = 16, 64
NCORES = 8
TS = 128  # s-tile (partition granularity)
TSL = 512  # t free-dim tile (one PSUM bank of fp32)
BF16 = ml_dtypes.bfloat16


def build_program(C_sz=C, T_sz=T, n_pairs=4, num_devices=1):
    import concourse.mybir as mybir
    from concourse import bacc
    from concourse.tile import TileContext

    dt = mybir.dt
    f32 = dt.float32
    bf16 = dt.bfloat16
    AF = mybir.ActivationFunctionType

    n_ct = C_sz // 128  # contraction tiles for projections
    n_qk = 2 * n_pairs  # qk o-tiles (128 channels each)
    VW = n_pairs * 2 * DK  # v channels (natural order)
    n_tt = T_sz // TS
    n_it = T_sz // TSL
    JPI = TSL // TS  # s-tiles per i-tile (4)
    OW = min(TSL, C_sz)  # output column tile width
    n_oh = C_sz // OW  # output column halves
    VEW = n_pairs * 2 * (DK + 1)  # v_ext width (65 per head)

    nc = bacc.Bacc(
        "TRN2",
        target_bir_lowering=False,
        debug=False,
        num_devices=num_devices,
    )

    xT_d = nc.dram_tensor("xT", [C_sz, T_sz], bf16, kind="ExternalInput").ap()
    wqk_d = nc.dram_tensor("wqkT", [C_sz, n_qk * 128], bf16, kind="ExternalInput").ap()
    wv_d = nc.dram_tensor("wvT", [C_sz, VW], bf16, kind="ExternalInput").ap()
    bqk_d = nc.dram_tensor("bqk", [128, n_qk], f32, kind="ExternalInput").ap()
    bv_d = nc.dram_tensor("bv", [1, VW], bf16, kind="ExternalInput").ap()
    wo_d = nc.dram_tensor("woT", [n_pairs * 128, C_sz], bf16, kind="ExternalInput").ap()
    mask_d = nc.dram_tensor("mask01", [128, 2 * TS], bf16, kind="ExternalInput").ap()
    out_d = nc.dram_tensor("out", [T_sz, C_sz], bf16, kind="ExternalOutput").ap()

    with TileContext(nc) as tc:
        with (
            tc.tile_pool(name="const", bufs=1) as const_pool,
            tc.tile_pool(name="big", bufs=1) as big_pool,
            tc.tile_pool(name="attn", bufs=10) as attn_pool,
            tc.tile_pool(name="rinv", bufs=6) as rinv_pool,
            tc.tile_pool(name="rbc", bufs=6) as rbc_pool,
            tc.tile_pool(name="outsb", bufs=6) as outsb_pool,
            tc.tile_pool(name="sc", bufs=2, space="PSUM") as sc_ps,
            tc.tile_pool(name="mm", bufs=4, space="PSUM") as mm_ps,
        ):
            # ---- SBUF tiles ----
            xT_sb = []
            wqk_sb = []
            wv_sb = []
            for ci in range(n_ct):
                xT_sb.append(
                    big_pool.tile([128, T_sz], bf16, tag=f"xT{ci}", name=f"xT{ci}")
                )
                wqk_sb.append(
                    big_pool.tile(
                        [128, n_qk * 128], bf16, tag=f"wqk{ci}", name=f"wqk{ci}"
                    )
                )
                wv_sb.append(big_pool.tile([128, VW], bf16, tag=f"wv{ci}", name=f"wv{ci}"))
            bqk_sb = const_pool.tile([128, n_qk], f32, tag="bqk", name="bqk")
            bv_sb = const_pool.tile([1, VW], bf16, tag="bv", name="bv")
            bv_bc = const_pool.tile([128, VW], bf16, tag="bv_bc", name="bv_bc")
            mask_sb = const_pool.tile([128, 2 * TS], bf16, tag="mask", name="mask")
            wo_sb = [
                big_pool.tile([128, C_sz], bf16, tag=f"wo{p}", name=f"wo{p}")
                for p in range(n_pairs)
            ]

            # HAM warmup: the PE clock-gate defaults to 1.2 GHz and reaches
            # 2.4 GHz only after ~3.4us of sustained matmul activity. The
            # first ~7us of the kernel are DMA-queue init with an idle PE;
            # dummy matmuls on scratch data warm the clock gate so the
            # DMA-paced ramp and everything after runs at full rate.
            # (Emitted before any other gpsimd work so the memset runs at t~0.)
            scratch_sb = const_pool.tile([128, TSL], bf16, tag="scr", name="scr")
            nc.vector.memset(scratch_sb[:], 0.0)
            warm_ps = mm_ps.tile([128, TSL], f32, tag="mm", name="mm")
            for _ in range(36):
                nc.tensor.matmul(
                    warm_ps[:],
                    lhsT=scratch_sb[:, 0:128],
                    rhs=scratch_sb[:],
                    start=True,
                    stop=True,
                    skip_group_check=True,
                )

            # ---- staged DMA issue: first-use order ----
            HQK = n_pairs * 128  # half of the qk o-range (ot 0..3)
            for ci in range(n_ct):
                nc.sync.dma_start(
                    xT_sb[ci][:, 0:TSL], xT_d[ci * 128 : (ci + 1) * 128, 0:TSL]
                )
                nc.sync.dma_start(
                    wqk_sb[ci][:, 0:HQK], wqk_d[ci * 128 : (ci + 1) * 128, 0:HQK]
                )
            nc.sync.dma_start(bqk_sb[:], bqk_d)
            for ci in range(n_ct):
                nc.sync.dma_start(wv_sb[ci][:], wv_d[ci * 128 : (ci + 1) * 128, :])
            nc.sync.dma_start(bv_sb[:], bv_d)
            nc.gpsimd.partition_broadcast(bv_bc[:], bv_sb[:])
            nc.sync.dma_start(mask_sb[:], mask_d)
            for ci in range(n_ct):
                nc.sync.dma_start(
                    wqk_sb[ci][:, HQK : n_qk * 128],
                    wqk_d[ci * 128 : (ci + 1) * 128, HQK : n_qk * 128],
                )
            for ci in range(n_ct):
                nc.sync.dma_start(
                    xT_sb[ci][:, TSL : 2 * TSL],
                    xT_d[ci * 128 : (ci + 1) * 128, TSL : 2 * TSL],
                )
            for p in range(n_pairs):
                nc.sync.dma_start(wo_sb[p][:], wo_d[p * 128 : (p + 1) * 128, :])
            for ii in range(2, n_it):
                for ci in range(n_ct):
                    nc.sync.dma_start(
                        xT_sb[ci][:, ii * TSL : (ii + 1) * TSL],
                        xT_d[ci * 128 : (ci + 1) * 128, ii * TSL : (ii + 1) * TSL],
                    )

            qkT_sb = [
                big_pool.tile([128, T_sz], bf16, tag=f"qkT{ot}", name=f"qkT{ot}")
                for ot in range(n_qk)
            ]
            vext_sb = [
                big_pool.tile([128, VEW], bf16, tag=f"vext{tt}", name=f"vext{tt}")
                for tt in range(n_tt)
            ]
            ctxT_sb = [
                big_pool.tile([128, T_sz], bf16, tag=f"ctxT{p}", name=f"ctxT{p}")
                for p in range(n_pairs)
            ]

            def qk_proj(ot, i):
                ps = mm_ps.tile([128, TSL], f32, tag="mm", name="mm")
                for ci in range(n_ct):
                    nc.tensor.matmul(
                        ps[:],
                        lhsT=wqk_sb[ci][:, ot * 128 : (ot + 1) * 128],
                        rhs=xT_sb[ci][:, i * TSL : (i + 1) * TSL],
                        start=(ci == 0),
                        stop=(ci == n_ct - 1),
                    )
                nc.vector.tensor_scalar_add(
                    qkT_sb[ot][:, i * TSL : (i + 1) * TSL],
                    ps[:],
                    bqk_sb[:, ot : ot + 1],
                )

            def v_proj(tt):
                ps = mm_ps.tile([128, VW], f32, tag="mm", name="mm")
                for ci in range(n_ct):
                    nc.tensor.matmul(
                        ps[:],
                        lhsT=xT_sb[ci][:, tt * TS : (tt + 1) * TS],
                        rhs=wv_sb[ci][:],
                        start=(ci == 0),
                        stop=(ci == n_ct - 1),
                    )
                vx = vext_sb[tt]
                vx3 = vx[:].rearrange("p (h e) -> p h e", e=DK + 1)
                nc.gpsimd.memset(vx3[:, :, DK : DK + 1], 1.0)
                nc.vector.scalar_tensor_tensor(
                    vx3[:, :, 0:DK],
                    ps[:].rearrange("p (h e) -> p h e", e=DK),
                    1.0,
                    bv_bc[:].rearrange("p (h e) -> p h e", e=DK),
                    op0=mybir.AluOpType.mult,
                    op1=mybir.AluOpType.add,
                )

            def out_proj(tt, oh):
                ps = mm_ps.tile([128, OW], f32, tag="mm", name="mm")
                for p in range(n_pairs):
                    nc.tensor.matmul(
                        ps[:],
                        lhsT=ctxT_sb[p][:, tt * TS : (tt + 1) * TS],
                        rhs=wo_sb[p][:, oh * OW : (oh + 1) * OW],
                        start=(p == 0),
                        stop=(p == n_pairs - 1),
                    )
                ob = outsb_pool.tile([128, OW], bf16, tag="outsb", name="outsb")
                nc.vector.tensor_copy(ob[:], ps[:])
                nc.sync.dma_start(
                    out_d[tt * TS : (tt + 1) * TS, oh * OW : (oh + 1) * OW],
                    ob[:],
                )

            # filler queues: must_q (next iter's projections, deadline = end
            # of this iter) is paced 3 per pair boundary; soft_q (prev iter's
            # out-projection) is rationed across pairs so every pair's
            # exp-latency bubbles get fill, with a reserve for the last
            # pair's normalize tail.
            must_q = deque()
            soft_q = deque()
            soft_allow = [0]

            def pump(q, n):
                for _ in range(n):
                    if q:
                        q.popleft()()

            def pump_soft(n=1):
                while n > 0 and soft_q and soft_allow[0] > 0:
                    soft_q.popleft()()
                    soft_allow[0] -= 1
                    n -= 1

            mask3 = mask_sb[:].rearrange("p (c w) -> p c w", c=2)

            def score_block(p, i, j):
                """Scores + exp (+ causal mask) for one s-block; returns the
                bf16 attn tile and its first live t column."""
                qt, kt = qkT_sb[2 * p], qkT_sb[2 * p + 1]
                diag = j >= JPI * i
                pi = j - JPI * i if diag else 0
                t0 = pi * TS  # first causally-live t column in this block
                ps = sc_ps.tile([128, 2 * TSL], f32, tag="sc", name="sc")
                nc.tensor.matmul(
                    ps[:, t0:TSL],
                    lhsT=kt[0:64, j * TS : (j + 1) * TS],
                    rhs=qt[0:64, i * TSL + t0 : (i + 1) * TSL],
                    start=True,
                    stop=True,
                    skip_group_check=True,
                )
                nc.tensor.matmul(
                    ps[:, TSL + t0 : 2 * TSL],
                    lhsT=kt[64:128, j * TS : (j + 1) * TS],
                    rhs=qt[64:128, i * TSL + t0 : (i + 1) * TSL],
                    start=True,
                    stop=True,
                    skip_group_check=True,
                )
                a = attn_pool.tile([128, 2 * TSL], bf16, tag="attn", name="attn")
                a3 = a[:].rearrange("p (c w) -> p c w", c=2)
                ps3 = ps[:].rearrange("p (c w) -> p c w", c=2)
                nc.scalar.activation(a3[:, :, t0:TSL], ps3[:, :, t0:TSL], AF.Exp)
                if diag:
                    nc.vector.tensor_mul(
                        a3[:, :, t0 : t0 + TS],
                        a3[:, :, t0 : t0 + TS],
                        mask3[:, :, :],
                    )
                return a, t0

            # score blocks pre-emitted ahead of their pair's body (the last
            # pair's tail overlaps the next pair's first exp latencies)
            heads = {}

            def emit_head(p, i, nh=2):
                heads[(p, i)] = [score_block(p, i, j) for j in range(min(nh, JPI * (i + 1)))]

            def attn_pair(p, i, splice=None):
                nj = JPI * (i + 1)
                pre = heads.pop((p, i), [])
                ctxA = mm_ps.tile([DK + 1, TSL], f32, tag="mm", name="mm")
                ctxB = mm_ps.tile([DK + 1, TSL], f32, tag="mm", name="mm")
                blocks = list(pre)
                for j in range(nj):
                    # keep the score pipeline one block ahead of ctx
                    while len(blocks) <= min(j + 1, nj - 1):
                        blocks.append(score_block(p, i, len(blocks)))
                    if j == nj - 1 and splice is not None:
                        splice()  # next pair's head: exps overlap our tail
                    a, t0 = blocks[j]
                    pump_soft(1)
                    nc.tensor.matmul(
                        ctxA[:, t0:TSL],
                        lhsT=vext_sb[j][:, (2 * p) * (DK + 1) : (2 * p + 1) * (DK + 1)],
                        rhs=a[:, t0:TSL],
                        start=(j == 0),
                        stop=(j == nj - 1),
                    )
                    nc.tensor.matmul(
                        ctxB[:, t0:TSL],
                        lhsT=vext_sb[j][
                            :, (2 * p + 1) * (DK + 1) : (2 * p + 2) * (DK + 1)
                        ],
                        rhs=a[:, TSL + t0 : 2 * TSL],
                        start=(j == 0),
                        stop=(j == nj - 1),
                    )
                isl = slice(i * TSL, (i + 1) * TSL)
                for cps, rows in ((ctxA, slice(0, 64)), (ctxB, slice(64, 128))):
                    # custom-DVE ops misread PSUM on hw: bounce rowsum via SBUF
                    rs = rinv_pool.tile([1, TSL], f32, tag="rsum", name="rsum")
                    nc.vector.tensor_copy(rs[:], cps[DK : DK + 1, :])
                    r = rinv_pool.tile([1, TSL], f32, tag="rinv", name="rinv")
                    nc.vector.reciprocal_approx_fast(r[:], rs[:])
                    rbc = rbc_pool.tile([DK, TSL], f32, tag="rbc", name="rbc")
                    nc.gpsimd.partition_broadcast(rbc[:], r[:])
                    nc.vector.tensor_mul(ctxT_sb[p][rows, isl], cps[0:DK, :], rbc[:])

            # ---- main schedule ----
            for ot in range(n_qk // 2):
                qk_proj(ot, 0)
            for tt in range(JPI):
                v_proj(tt)
            emit_head(0, 0)
            for i in range(n_it):
                if i + 1 < n_it:
                    for ot in range(n_qk):
                        must_q.append(lambda ot=ot, i=i: qk_proj(ot, i + 1))
                    for tt in range(JPI * (i + 1), JPI * (i + 2)):
                        must_q.append(lambda tt=tt: v_proj(tt))
                if i > 0:
                    for tt in range(JPI * (i - 1), JPI * i):
                        for oh in range(n_oh):
                            soft_q.append(lambda tt=tt, oh=oh: out_proj(tt, oh))
                last_it = i == n_it - 1
                for p in range(n_pairs):
                    if i == 0 and p in (1, 2):
                        # qkT for pairs 2/3 must be emitted before their use
                        # (pair p+1's head is spliced into pair p's body)
                        qk_proj(2 * p + 2, 0)
                        qk_proj(2 * p + 3, 0)
                    if i == 0:
                        if p > 0:
                            pump(must_q, 4)
                    else:
                        pump(must_q, 3)
                    if last_it:
                        # hold fill back for the final normalize latency
                        soft_allow[0] = 1
                    else:
                        soft_allow[0] = -(-len(soft_q) // max(n_pairs - p, 1))
                    if p + 1 < n_pairs:
                        nxt = (p + 1, i)
                    elif i + 1 < n_it:
                        nxt = (0, i + 1)
                    else:
                        nxt = None
                    attn_pair(
                        p,
                        i,
                        splice=(lambda nxt=nxt: emit_head(*nxt)) if nxt else None,
                    )
                # any qk/v leftovers must land before the next iteration
                pump(must_q, len(must_q))
                if not last_it:
                    soft_allow[0] = len(soft_q)
                    pump_soft(len(soft_q))
            # iter-3 reserve: fill the last pair's normalize latency
            soft_allow[0] = len(soft_q)
            pump_soft(len(soft_q))
            for tt in range(JPI * (n_it - 1), JPI * n_it):
                for oh in range(n_oh):
                    out_proj(tt, oh)

    nc.compile()
    return nc


def make_mask01(ts=TS):
    """[128, 2*ts] bf16 {0,1}: cell (s, t) = 0 iff s > t, two copies."""
    s = np.arange(128)[:, None]
    t = np.arange(ts)[None, :]
    m = np.where(s > t, 0.0, 1.0).astype(np.float32)
    return np.concatenate([m, m], axis=1)


def make_core_inputs(x_b, W_qkv, b_qkv, W_out, heads, C_sz=C, T_sz=T):
    """Build the per-core input map (numpy, host-side)."""
    n_pairs = len(heads) // 2
    n_qk = 2 * n_pairs
    VW = len(heads) * DK
    xT = np.ascontiguousarray(x_b.T).astype(BF16)
    wqk = np.empty((C_sz, n_qk * 128), np.float32)
    bqk = np.empty((128, n_qk), np.float32)
    wv = np.empty((C_sz, VW), np.float32)
    bv = np.empty((1, VW), np.float32)
    wo = np.empty((n_pairs * 128, C_sz), np.float32)
    for p in range(n_pairs):
        hA, hB = heads[2 * p], heads[2 * p + 1]
        # q tile (scaled by 1/sqrt(dk)=1/8), k tile
        for half, h in ((0, hA), (1, hB)):
            r0 = h * 3 * DK
            wqk[:, 2 * p * 128 + half * 64 : 2 * p * 128 + half * 64 + 64] = (
                W_qkv[r0 : r0 + DK].T / math.sqrt(DK)
            )
            bqk[half * 64 : half * 64 + 64, 2 * p] = b_qkv[r0 : r0 + DK] / math.sqrt(DK)
            wqk[:, (2 * p + 1) * 128 + half * 64 : (2 * p + 1) * 128 + half * 64 + 64] = (
                W_qkv[r0 + DK : r0 + 2 * DK].T
            )
            bqk[half * 64 : half * 64 + 64, 2 * p + 1] = b_qkv[r0 + DK : r0 + 2 * DK]
            wo[p * 128 + half * 64 : p * 128 + half * 64 + 64, :] = W_out[
                :, h * DK : (h + 1) * DK
            ].T
    for hh, h in enumerate(heads):
        r0 = h * 3 * DK + 2 * DK
        wv[:, hh * DK : (hh + 1) * DK] = W_qkv[r0 : r0 + DK].T
        bv[0, hh * DK : (hh + 1) * DK] = b_qkv[r0 : r0 + DK]
    return {
        "xT": xT,
        "wqkT": wqk.astype(BF16),
        "wvT": wv.astype(BF16),
        "bqk": bqk.astype(np.float32),
        "bv": bv.astype(BF16),
        "woT": wo.astype(BF16),
        "mask01": make_mask01().astype(BF16),
    }


_NC_CACHE = {}


def kernel(x, W_qkv, b_qkv, W_out, b_out, _trace=False):
    x = np.asarray(x, dtype=np.float32)
    W_qkv = np.asarray(W_qkv, dtype=np.float32)
    b_qkv = np.asarray(b_qkv, dtype=np.float32)
    W_out = np.asarray(W_out, dtype=np.float32)
    b_out = np.asarray(b_out, dtype=np.float32)

    from concourse.bass_utils import run_bass_kernel_spmd

    key = ("full", C, T, 4)
    if key not in _NC_CACHE:
        _NC_CACHE[key] = build_program(C, T, n_pairs=4, num_devices=1)
    nc = _NC_CACHE[key]

    in_maps = []
    for core in range(NCORES):
        b, hg = divmod(core, 2)
        heads = list(range(hg * 8, hg * 8 + 8))
        in_maps.append(make_core_inputs(x[b], W_qkv, b_qkv, W_out, heads))

    res = run_bass_kernel_spmd(nc, in_maps, list(range(NCORES)), trace=_trace)
    kernel._last_results = res

    out = np.broadcast_to(b_out, (B, T, C)).astype(np.float32).copy()
    for core in range(NCORES):
        b = core // 2
        out[b] += np.asarray(res.results[core]["out"], dtype=np.float32)
    return out


# revision 26
# speedup vs baseline: 1.1030x; 1.0021x over previous
"""Causal self-attention Trainium2 kernel (B=4, T=2048, D=1024, H=16).

Sharding: 8 cores = 4 batches x 2 head-groups (8 heads each). Each core
computes its batch's qkv projection restricted to its 8 heads, causal
attention for those heads, and a partial out-projection over its 512 ctx
channels. Host sums the two partials per batch and adds b_out.

Per-core layout choices (all matmuls bf16 with fp32 PSUM accumulation):
  - xT [C, T]: channels on partitions (contraction dim for projections).
  - qkT: per head-pair p, a q-tile [128, T] (head A rows 0:64, head B rows
    64:128) and a k-tile [128, T]. Produced directly transposed by making
    W the stationary operand. The 1/sqrt(dk) scale is folded into Wq/bq.
  - scoresT[s, t] blocks [128, 512]: lhsT=kT (K=64 rows), rhs=qT. Heads A/B
    are row-packed (PE row groups 0:64 / 64:128) and run concurrently.
    Diagonal blocks only compute the causally needed t-range.
  - causal mask: after exp, the diagonal 128x128 squares are multiplied
    in-place (DVE) by a {0,1} strict-lower-triangular bf16 mask.
  - softmax: no max-subtraction (scores are within +-10 by construction),
    exp on ScalarE PSUM->SBUF bf16.
  - ctx: v stored naturally [s, d] with a ones column appended per head
    (v_ext [128, 8*65]); lhsT=v_ext (M=65) so PSUM row 64 accumulates the
    softmax denominator. Normalize = reciprocal_approx_fast + gpsimd
    partition_broadcast + DVE mul into the bf16 ctxT copy.
  - out projection: ctxT pair-tiles [128, T] are the stationary operand
    against W_outT; b_out is added on the host (once per batch).

Scheduling: the per-engine instruction streams execute strictly in
emission order, so filler matmuls (next iteration's projections and the
previous iteration's out-projection) are pumped from deques into the
exact emission points where the PE would otherwise stall on ScalarE exp
(pair starts and the steady j-loop). ScalarE runs exp only; bias adds
and PSUM evacuations run on DVE. DMAs are staged in first-use order so
compute starts ~3us in.
"""

import math
from collections import deque

import numpy as np
import ml_dtypes

B, T, C = 4, 2048, 1024
H, DK = 16, 64
NCORES = 8
TS = 128  # s-tile (partition granularity)
TSL = 512  # t free-dim tile (one PSUM bank of fp32)
BF16 = ml_dtypes.bfloat16


def build_program(C_sz=C, T_sz=T, n_pairs=4, num_devices=1):
    import concourse.mybir as mybir
    from concourse import bacc
    from concourse.tile import TileContext

    dt = mybir.dt
    f32 = dt.float32
    bf16 = dt.bfloat16
    AF = mybir.ActivationFunctionType

    n_ct = C_sz // 128  # contraction tiles for projections
    n_qk = 2 * n_pairs  # qk o-tiles (128 channels each)
    VW = n_pairs * 2 * DK  # v channels (natural order)
    n_tt = T_sz // TS
    n_it = T_sz // TSL
    JPI = TSL // TS  # s-tiles per i-tile (4)
    OW = min(TSL, C_sz)  # output column tile width
    n_oh = C_sz // OW  # output column halves
    VEW = n_pairs * 2 * (DK + 1)  # v_ext width (65 per head)

    nc = bacc.Bacc(
        "TRN2",
        target_bir_lowering=False,
        debug=False,
        num_devices=num_devices,
    )

    xT_d = nc.dram_tensor("xT", [C_sz, T_sz], bf16, kind="ExternalInput").ap()
    wqk_d = nc.dram_tensor("wqkT", [C_sz, n_qk * 128], bf16, kind="ExternalInput").ap()
    wv_d = nc.dram_tensor("wvT", [C_sz, VW], bf16, kind="ExternalInput").ap()
    bqk_d = nc.dram_tensor("bqk", [128, n_qk], f32, kind="ExternalInput").ap()
    bv_d = nc.dram_tensor("bv", [1, VW], bf16, kind="ExternalInput").ap()
    wo_d = nc.dram_tensor("woT", [n_pairs * 128, C_sz], bf16, kind="ExternalInput").ap()
    mask_d = nc.dram_tensor("mask01", [128, 2 * TS], bf16, kind="ExternalInput").ap()
    out_d = nc.dram_tensor("out", [T_sz, C_sz], bf16, kind="ExternalOutput").ap()

    with TileContext(nc) as tc:
        with (
            tc.tile_pool(name="const", bufs=1) as const_pool,
            tc.tile_pool(name="big", bufs=1) as big_pool,
            tc.tile_pool(name="attn", bufs=10) as attn_pool,
            tc.tile_pool(name="rinv", bufs=6) as rinv_pool,
            tc.tile_pool(name="rbc", bufs=6) as rbc_pool,
            tc.tile_pool(name="outsb", bufs=6) as outsb_pool,
            tc.tile_pool(name="sc", bufs=2, space="PSUM") as sc_ps,
            tc.tile_pool(name="mm", bufs=4, space="PSUM") as mm_ps,
        ):
            # ---- SBUF tiles ----
            xT_sb = []
            wqk_sb = []
            wv_sb = []
            for ci in range(n_ct):
                xT_sb.append(
                    big_pool.tile([128, T_sz], bf16, tag=f"xT{ci}", name=f"xT{ci}")
                )
                wqk_sb.append(
                    big_pool.tile(
                        [128, n_qk * 128], bf16, tag=f"wqk{ci}", name=f"wqk{ci}"
                    )
                )
                wv_sb.append(big_pool.tile([128, VW], bf16, tag=f"wv{ci}", name=f"wv{ci}"))
            bqk_sb = const_pool.tile([128, n_qk], f32, tag="bqk", name="bqk")
            bv_sb = const_pool.tile([1, VW], bf16, tag="bv", name="bv")
            bv_bc = const_pool.tile([128, VW], bf16, tag="bv_bc", name="bv_bc")
            mask_sb = const_pool.tile([128, 2 * TS], bf16, tag="mask", name="mask")
            wo_sb = [
                big_pool.tile([128, C_sz], bf16, tag=f"wo{p}", name=f"wo{p}")
                for p in range(n_pairs)
            ]

            # HAM warmup: the PE clock-gate defaults to 1.2 GHz and reaches
            # 2.4 GHz only after ~3.4us of sustained matmul activity. The
            # first ~7us of the kernel are DMA-queue init with an idle PE;
            # dummy matmuls on scratch data warm the clock gate so the
            # DMA-paced ramp and everything after runs at full rate.
            # (Emitted before any other gpsimd work so the memset runs at t~0.)
            scratch_sb = const_pool.tile([128, TSL], bf16, tag="scr", name="scr")
            nc.vector.memset(scratch_sb[:], 0.0)
            warm_ps = mm_ps.tile([128, TSL], f32, tag="mm", name="mm")
            for _ in range(36):
                nc.tensor.matmul(
                    warm_ps[:],
                    lhsT=scratch_sb[:, 0:128],
                    rhs=scratch_sb[:],
                    start=True,
                    stop=True,
                    skip_group_check=True,
                )

            # ---- staged DMA issue: first-use order ----
            HQK = n_pairs * 128  # half of the qk o-range (ot 0..3)
            for ci in range(n_ct):
                nc.sync.dma_start(
                    xT_sb[ci][:, 0:TSL], xT_d[ci * 128 : (ci + 1) * 128, 0:TSL]
                )
                nc.sync.dma_start(
                    wqk_sb[ci][:, 0:HQK], wqk_d[ci * 128 : (ci + 1) * 128, 0:HQK]
                )
            nc.sync.dma_start(bqk_sb[:], bqk_d)
            for ci in range(n_ct):
                nc.sync.dma_start(wv_sb[ci][:], wv_d[ci * 128 : (ci + 1) * 128, :])
            nc.sync.dma_start(bv_sb[:], bv_d)
            nc.gpsimd.partition_broadcast(bv_bc[:], bv_sb[:])
            nc.sync.dma_start(mask_sb[:], mask_d)
            for ci in range(n_ct):
                nc.sync.dma_start(
                    wqk_sb[ci][:, HQK : n_qk * 128],
                    wqk_d[ci * 128 : (ci + 1) * 128, HQK : n_qk * 128],
                )
            for ci in range(n_ct):
                nc.sync.dma_start(
                    xT_sb[ci][:, TSL : 2 * TSL],
                    xT_d[ci * 128 : (ci + 1) * 128, TSL : 2 * TSL],
                )
            for p in range(n_pairs):
                nc.sync.dma_start(wo_sb[p][:], wo_d[p * 128 : (p + 1) * 128, :])
            for ii in range(2, n_it):
                for ci in range(n_ct):
                    nc.sync.dma_start(
                        xT_sb[ci][:, ii * TSL : (ii + 1) * TSL],
                        xT_d[ci * 128 : (ci + 1) * 128, ii * TSL : (ii + 1) * TSL],
                    )

            qkT_sb = [
                big_pool.tile([128, T_sz], bf16, tag=f"qkT{ot}", name=f"qkT{ot}")
                for ot in range(n_qk)
            ]
            vext_sb = [
                big_pool.tile([128, VEW], bf16, tag=f"vext{tt}", name=f"vext{tt}")
                for tt in range(n_tt)
            ]
            ctxT_sb = [
                big_pool.tile([128, T_sz], bf16, tag=f"ctxT{p}", name=f"ctxT{p}")
                for p in range(n_pairs)
            ]

            def qk_proj(ot, i):
                ps = mm_ps.tile([128, TSL], f32, tag="mm", name="mm")
                for ci in range(n_ct):
                    nc.tensor.matmul(
                        ps[:],
                        lhsT=wqk_sb[ci][:, ot * 128 : (ot + 1) * 128],
                        rhs=xT_sb[ci][:, i * TSL : (i + 1) * TSL],
                        start=(ci == 0),
                        stop=(ci == n_ct - 1),
                    )
                nc.vector.tensor_scalar_add(
                    qkT_sb[ot][:, i * TSL : (i + 1) * TSL],
                    ps[:],
                    bqk_sb[:, ot : ot + 1],
                )

            def v_proj(tt):
                ps = mm_ps.tile([128, VW], f32, tag="mm", name="mm")
                for ci in range(n_ct):
                    nc.tensor.matmul(
                        ps[:],
                        lhsT=xT_sb[ci][:, tt * TS : (tt + 1) * TS],
                        rhs=wv_sb[ci][:],
                        start=(ci == 0),
                        stop=(ci == n_ct - 1),
                    )
                vx = vext_sb[tt]
                vx3 = vx[:].rearrange("p (h e) -> p h e", e=DK + 1)
                nc.gpsimd.memset(vx3[:, :, DK : DK + 1], 1.0)
                nc.vector.scalar_tensor_tensor(
                    vx3[:, :, 0:DK],
                    ps[:].rearrange("p (h e) -> p h e", e=DK),
                    1.0,
                    bv_bc[:].rearrange("p (h e) -> p h e", e=DK),
                    op0=mybir.AluOpType.mult,
                    op1=mybir.AluOpType.add,
                )

            def out_proj(tt, oh):
                ps = mm_ps.tile([128, OW], f32, tag="mm", name="mm")
                for p in range(n_pairs):
                    nc.tensor.matmul(
                        ps[:],
                        lhsT=ctxT_sb[p][:, tt * TS : (tt + 1) * TS],
                        rhs=wo_sb[p][:, oh * OW : (oh + 1) * OW],
                        start=(p == 0),
                        stop=(p == n_pairs - 1),
                    )
                ob = outsb_pool.tile([128, OW], bf16, tag="outsb", name="outsb")
                nc.vector.tensor_copy(ob[:], ps[:])
                nc.sync.dma_start(
                    out_d[tt * TS : (tt + 1) * TS, oh * OW : (oh + 1) * OW],
                    ob[:],
                )

            # filler queues: must_q (next iter's projections, deadline = end
            # of this iter) is paced 3 per pair boundary; soft_q (prev iter's
            # out-projection) is rationed across pairs so every pair's
            # exp-latency bubbles get fill, with a reserve for the last
            # pair's normalize tail.
            must_q = deque()
            soft_q = deque()
            soft_allow = [0]

            def pump(q, n):
                for _ in range(n):
                    if q:
                        q.popleft()()

            def pump_soft(n=1):
                while n > 0 and soft_q and soft_allow[0] > 0:
                    soft_q.popleft()()
                    soft_allow[0] -= 1
                    n -= 1

            mask3 = mask_sb[:].rearrange("p (c w) -> p c w", c=2)

            def score_block(p, i, j):
                """Scores + exp (+ causal mask) for one s-block; returns the
                bf16 attn tile and its first live t column."""
                qt, kt = qkT_sb[2 * p], qkT_sb[2 * p + 1]
                diag = j >= JPI * i
                pi = j - JPI * i if diag else 0
                t0 = pi * TS  # first causally-live t column in this block
                ps = sc_ps.tile([128, 2 * TSL], f32, tag="sc", name="sc")
                nc.tensor.matmul(
                    ps[:, t0:TSL],
                    lhsT=kt[0:64, j * TS : (j + 1) * TS],
                    rhs=qt[0:64, i * TSL + t0 : (i + 1) * TSL],
                    start=True,
                    stop=True,
                    skip_group_check=True,
                )
                nc.tensor.matmul(
                    ps[:, TSL + t0 : 2 * TSL],
                    lhsT=kt[64:128, j * TS : (j + 1) * TS],
                    rhs=qt[64:128, i * TSL + t0 : (i + 1) * TSL],
                    start=True,
                    stop=True,
                    skip_group_check=True,
                )
                a = attn_pool.tile([128, 2 * TSL], bf16, tag="attn", name="attn")
                a3 = a[:].rearrange("p (c w) -> p c w", c=2)
                ps3 = ps[:].rearrange("p (c w) -> p c w", c=2)
                nc.scalar.activation(a3[:, :, t0:TSL], ps3[:, :, t0:TSL], AF.Exp)
                if diag:
                    nc.vector.tensor_mul(
                        a3[:, :, t0 : t0 + TS],
                        a3[:, :, t0 : t0 + TS],
                        mask3[:, :, :],
                    )
                return a, t0

            # score blocks pre-emitted ahead of their pair's body (the last
            # pair's tail overlaps the next pair's first exp latencies)
            heads = {}

            def emit_head(p, i, nh=2):
                heads[(p, i)] = [score_block(p, i, j) for j in range(min(nh, JPI * (i + 1)))]

            def attn_pair(p, i, splice=None):
                nj = JPI * (i + 1)
                pre = heads.pop((p, i), [])
                ctxA = mm_ps.tile([DK + 1, TSL], f32, tag="mm", name="mm")
                ctxB = mm_ps.tile([DK + 1, TSL], f32, tag="mm", name="mm")
                blocks = list(pre)
                for j in range(nj):
                    # keep the score pipeline one block ahead of ctx
                    while len(blocks) <= min(j + 1, nj - 1):
                        blocks.append(score_block(p, i, len(blocks)))
                    if j == nj - 1 and splice is not None:
                        splice()  # next pair's head: exps overlap our tail
                    a, t0 = blocks[j]
                    pump_soft(1)
                    nc.tensor.matmul(
                        ctxA[:, t0:TSL],
                        lhsT=vext_sb[j][:, (2 * p) * (DK + 1) : (2 * p + 1) * (DK + 1)],
                        rhs=a[:, t0:TSL],
                        start=(j == 0),
                        stop=(j == nj - 1),
                    )
                    nc.tensor.matmul(
                        ctxB[:, t0:TSL],
                        lhsT=vext_sb[j][
                            :, (2 * p + 1) * (DK + 1) : (2 * p + 2) * (DK + 1)
                        ],
                        rhs=a[:, TSL + t0 : 2 * TSL],
                        start=(j == 0),
                        stop=(j == nj - 1),
                    )
                isl = slice(i * TSL, (i + 1) * TSL)
                for cps, rows in ((ctxA, slice(0, 64)), (ctxB, slice(64, 128))):
                    # custom-DVE ops misread PSUM on hw: bounce rowsum via SBUF
                    rs = rinv_pool.tile([1, TSL], f32, tag="rsum", name="rsum")
                    nc.vector.tensor_copy(rs[:], cps[DK : DK + 1, :])
                    r = rinv_pool.tile([1, TSL], f32, tag="rinv", name="rinv")
                    nc.vector.reciprocal_approx_fast(r[:], rs[:])
                    rbc = rbc_pool.tile([DK, TSL], f32, tag="rbc", name="rbc")
                    nc.gpsimd.partition_broadcast(rbc[:], r[:])
                    nc.vector.tensor_mul(ctxT_sb[p][rows, isl], cps[0:DK, :], rbc[:])

            # ---- main schedule ----
            for ot in range(n_qk // 2):
                qk_proj(ot, 0)
            for tt in range(JPI):
                v_proj(tt)
            emit_head(0, 0)
            for i in range(n_it):
                if i + 1 < n_it:
                    for ot in range(n_qk):
                        must_q.append(lambda ot=ot, i=i: qk_proj(ot, i + 1))
                    for tt in range(JPI * (i + 1), JPI * (i + 2)):
                        must_q.append(lambda tt=tt: v_proj(tt))
                if i > 0:
                    for tt in range(JPI * (i - 1), JPI * i):
                        for oh in range(n_oh):
                            soft_q.append(lambda tt=tt, oh=oh: out_proj(tt, oh))
                last_it = i == n_it - 1
                for p in range(n_pairs):
                    if i == 0 and p in (1, 2):
                        # qkT for pairs 2/3 must be emitted before their use
                        # (pair p+1's head is spliced into pair p's body)
                        qk_proj(2 * p + 2, 0)
                        qk_proj(2 * p + 3, 0)
                    if i == 0:
                        if p > 0:
                            pump(must_q, 4)
                    else:
                        pump(must_q, 3)
                    if last_it:
                        # hold fill back for the final normalize latency (it
                        # also keeps the PE clock-gate warm for the last
                        # out-projections)
                        soft_allow[0] = 1 if p < 2 else 0
                    else:
                        soft_allow[0] = -(-len(soft_q) // max(n_pairs - p, 1))
                    if p + 1 < n_pairs:
                        nxt = (p + 1, i)
                    elif i + 1 < n_it:
                        nxt = (0, i + 1)
                    else:
                        nxt = None
                    attn_pair(
                        p,
                        i,
                        splice=(lambda nxt=nxt: emit_head(*nxt)) if nxt else None,
                    )
                # any qk/v leftovers must land before the next iteration
                pump(must_q, len(must_q))
                if not last_it:
                    soft_allow[0] = len(soft_q)
                    pump_soft(len(soft_q))
            # iter-3 reserve: fill the last pair's normalize latency
            soft_allow[0] = len(soft_q)
            pump_soft(len(soft_q))
            for tt in range(JPI * (n_it - 1), JPI * n_it):
                for oh in range(n_oh):
                    out_proj(tt, oh)

    nc.compile()
    return nc


def make_mask01(ts=TS):
    """[128, 2*ts] bf16 {0,1}: cell (s, t) = 0 iff s > t, two copies."""
    s = np.arange(128)[:, None]
    t = np.arange(ts)[None, :]
    m = np.where(s > t, 0.0, 1.0).astype(np.float32)
    return np.concatenate([m, m], axis=1)


def make_core_inputs(x_b, W_qkv, b_qkv, W_out, heads, C_sz=C, T_sz=T):
    """Build the per-core input map (numpy, host-side)."""
    n_pairs = len(heads) // 2
    n_qk = 2 * n_pairs
    VW = len(heads) * DK
    xT = np.ascontiguousarray(x_b.T).astype(BF16)
    wqk = np.empty((C_sz, n_qk * 128), np.float32)
    bqk = np.empty((128, n_qk), np.float32)
    wv = np.empty((C_sz, VW), np.float32)
    bv = np.empty((1, VW), np.float32)
    wo = np.empty((n_pairs * 128, C_sz), np.float32)
    for p in range(n_pairs):
        hA, hB = heads[2 * p], heads[2 * p + 1]
        # q tile (scaled by 1/sqrt(dk)=1/8), k tile
        for half, h in ((0, hA), (1, hB)):
            r0 = h * 3 * DK
            wqk[:, 2 * p * 128 + half * 64 : 2 * p * 128 + half * 64 + 64] = (
                W_qkv[r0 : r0 + DK].T / math.sqrt(DK)
            )
            bqk[half * 64 : half * 64 + 64, 2 * p] = b_qkv[r0 : r0 + DK] / math.sqrt(DK)
            wqk[:, (2 * p + 1) * 128 + half * 64 : (2 * p + 1) * 128 + half * 64 + 64] = (
                W_qkv[r0 + DK : r0 + 2 * DK].T
            )
            bqk[half * 64 : half * 64 + 64, 2 * p + 1] = b_qkv[r0 + DK : r0 + 2 * DK]
            wo[p * 128 + half * 64 : p * 128 + half * 64 + 64, :] = W_out[
                :, h * DK : (h + 1) * DK
            ].T
    for hh, h in enumerate(heads):
        r0 = h * 3 * DK + 2 * DK
        wv[:, hh * DK : (hh + 1) * DK] = W_qkv[r0 : r0 + DK].T
        bv[0, hh * DK : (hh + 1) * DK] = b_qkv[r0 : r0 + DK]
    return {
        "xT": xT,
        "wqkT": wqk.astype(BF16),
        "wvT": wv.astype(BF16),
        "bqk": bqk.astype(np.float32),
        "bv": bv.astype(BF16),
        "woT": wo.astype(BF16),
        "mask01": make_mask01().astype(BF16),
    }


_NC_CACHE = {}


def kernel(x, W_qkv, b_qkv, W_out, b_out, _trace=False):
    x = np.asarray(x, dtype=np.float32)
    W_qkv = np.asarray(W_qkv, dtype=np.float32)
    b_qkv = np.asarray(b_qkv, dtype=np.float32)
    W_out = np.asarray(W_out, dtype=np.float32)
    b_out = np.asarray(b_out, dtype=np.float32)

    from concourse.bass_utils import run_bass_kernel_spmd

    key = ("full", C, T, 4)
    if key not in _NC_CACHE:
        _NC_CACHE[key] = build_program(C, T, n_pairs=4, num_devices=1)
    nc = _NC_CACHE[key]

    in_maps = []
    for core in range(NCORES):
        b, hg = divmod(core, 2)
        heads = list(range(hg * 8, hg * 8 + 8))
        in_maps.append(make_core_inputs(x[b], W_qkv, b_qkv, W_out, heads))

    res = run_bass_kernel_spmd(nc, in_maps, list(range(NCORES)), trace=_trace)
    kernel._last_results = res

    out = np.broadcast_to(b_out, (B, T, C)).astype(np.float32).copy()
    for core in range(NCORES):
        b = core // 2
        out[b] += np.asarray(res.results[core]["out"], dtype=np.float32)
    return out


# revision 29
# speedup vs baseline: 1.1109x; 1.0071x over previous
"""Causal self-attention Trainium2 kernel (B=4, T=2048, D=1024, H=16).

Sharding: 8 cores = 4 batches x 2 head-groups (8 heads each). Each core
computes its batch's qkv projection restricted to its 8 heads, causal
attention for those heads, and a partial out-projection over its 512 ctx
channels. Host sums the two partials per batch and adds b_out.

Per-core layout choices (all matmuls bf16 with fp32 PSUM accumulation):
  - xT [C, T]: channels on partitions (contraction dim for projections).
  - qkT: per head-pair p, a q-tile [128, T] (head A rows 0:64, head B rows
    64:128) and a k-tile [128, T]. Produced directly transposed by making
    W the stationary operand. The 1/sqrt(dk) scale is folded into Wq/bq.
  - scoresT[s, t] blocks [128, 512]: lhsT=kT (K=64 rows), rhs=qT. Heads A/B
    are row-packed (PE row groups 0:64 / 64:128) and run concurrently.
    Diagonal blocks only compute the causally needed t-range.
  - causal mask: after exp, the diagonal 128x128 squares are multiplied
    in-place (DVE) by a {0,1} strict-lower-triangular bf16 mask.
  - softmax: no max-subtraction (scores are within +-10 by construction),
    exp on ScalarE PSUM->SBUF bf16.
  - ctx: v stored naturally [s, d] with a ones column appended per head
    (v_ext [128, 8*65]); lhsT=v_ext (M=65) so PSUM row 64 accumulates the
    softmax denominator. Normalize = reciprocal_approx_fast + gpsimd
    partition_broadcast + DVE mul into the bf16 ctxT copy.
  - out projection: ctxT pair-tiles [128, T] are the stationary operand
    against W_outT; b_out is added on the host (once per batch).

Scheduling: the per-engine instruction streams execute strictly in
emission order, so filler matmuls (next iteration's projections and the
previous iteration's out-projection) are pumped from deques into the
exact emission points where the PE would otherwise stall on ScalarE exp
(pair starts and the steady j-loop). ScalarE runs exp only; bias adds
and PSUM evacuations run on DVE. DMAs are staged in first-use order so
compute starts ~3us in.
"""

import math
from collections import deque

import numpy as np
import ml_dtypes

B, T, C = 4, 2048, 1024
H, DK = 16, 64
NCORES = 8
TS = 128  # s-tile (partition granularity)
TSL = 512  # t free-dim tile (one PSUM bank of fp32)
BF16 = ml_dtypes.bfloat16


def build_program(C_sz=C, T_sz=T, n_pairs=4, num_devices=1):
    import concourse.mybir as mybir
    from concourse import bacc
    from concourse.tile import TileContext

    dt = mybir.dt
    f32 = dt.float32
    bf16 = dt.bfloat16
    AF = mybir.ActivationFunctionType

    n_ct = C_sz // 128  # contraction tiles for projections
    n_qk = 2 * n_pairs  # qk o-tiles (128 channels each)
    VW = n_pairs * 2 * DK  # v channels (natural order)
    n_tt = T_sz // TS
    n_it = T_sz // TSL
    JPI = TSL // TS  # s-tiles per i-tile (4)
    OW = min(TSL, C_sz)  # output column tile width
    n_oh = C_sz // OW  # output column halves
    VEW = n_pairs * 2 * (DK + 1)  # v_ext width (65 per head)

    nc = bacc.Bacc(
        "TRN2",
        target_bir_lowering=False,
        debug=False,
        num_devices=num_devices,
    )

    xT_d = nc.dram_tensor("xT", [C_sz, T_sz], bf16, kind="ExternalInput").ap()
    wqk_d = nc.dram_tensor("wqkT", [C_sz, n_qk * 128], bf16, kind="ExternalInput").ap()
    wv_d = nc.dram_tensor("wvT", [C_sz, VW], bf16, kind="ExternalInput").ap()
    bqk_d = nc.dram_tensor("bqk", [128, n_qk], f32, kind="ExternalInput").ap()
    bv_d = nc.dram_tensor("bv", [1, VW], bf16, kind="ExternalInput").ap()
    wo_d = nc.dram_tensor("woT", [n_pairs * 128, C_sz], bf16, kind="ExternalInput").ap()
    mask_d = nc.dram_tensor("mask01", [128, 2 * TS], bf16, kind="ExternalInput").ap()
    out_d = nc.dram_tensor("out", [T_sz, C_sz], bf16, kind="ExternalOutput").ap()

    with TileContext(nc) as tc:
        with (
            tc.tile_pool(name="const", bufs=1) as const_pool,
            tc.tile_pool(name="big", bufs=1) as big_pool,
            tc.tile_pool(name="attn", bufs=10) as attn_pool,
            tc.tile_pool(name="rinv", bufs=6) as rinv_pool,
            tc.tile_pool(name="rbc", bufs=6) as rbc_pool,
            tc.tile_pool(name="outsb", bufs=6) as outsb_pool,
            tc.tile_pool(name="sc", bufs=2, space="PSUM") as sc_ps,
            tc.tile_pool(name="mm", bufs=4, space="PSUM") as mm_ps,
        ):
            # ---- SBUF tiles ----
            xT_sb = []
            wqk_sb = []
            wv_sb = []
            for ci in range(n_ct):
                xT_sb.append(
                    big_pool.tile([128, T_sz], bf16, tag=f"xT{ci}", name=f"xT{ci}")
                )
                wqk_sb.append(
                    big_pool.tile(
                        [128, n_qk * 128], bf16, tag=f"wqk{ci}", name=f"wqk{ci}"
                    )
                )
                wv_sb.append(big_pool.tile([128, VW], bf16, tag=f"wv{ci}", name=f"wv{ci}"))
            bqk_sb = const_pool.tile([128, n_qk], f32, tag="bqk", name="bqk")
            bv_sb = const_pool.tile([1, VW], bf16, tag="bv", name="bv")
            bv_bc = const_pool.tile([128, VW], bf16, tag="bv_bc", name="bv_bc")
            mask_sb = const_pool.tile([128, 2 * TS], bf16, tag="mask", name="mask")
            wo_sb = [
                big_pool.tile([128, C_sz], bf16, tag=f"wo{p}", name=f"wo{p}")
                for p in range(n_pairs)
            ]

            # HAM warmup: the PE clock-gate defaults to 1.2 GHz and reaches
            # 2.4 GHz only after ~3.4us of sustained matmul activity. The
            # first ~7us of the kernel are DMA-queue init with an idle PE;
            # dummy matmuls on scratch data warm the clock gate so the
            # DMA-paced ramp and everything after runs at full rate.
            # (Emitted before any other gpsimd work so the memset runs at t~0.)
            scratch_sb = const_pool.tile([128, TSL], bf16, tag="scr", name="scr")
            nc.vector.memset(scratch_sb[:], 0.0)
            warm_ps = mm_ps.tile([128, TSL], f32, tag="mm", name="mm")
            for _ in range(36):
                nc.tensor.matmul(
                    warm_ps[:],
                    lhsT=scratch_sb[:, 0:128],
                    rhs=scratch_sb[:],
                    start=True,
                    stop=True,
                    skip_group_check=True,
                )

            # ---- staged DMA issue: first-use order ----
            HQK = n_pairs * 128  # half of the qk o-range (ot 0..3)
            for ci in range(n_ct):
                nc.sync.dma_start(
                    xT_sb[ci][:, 0:TSL], xT_d[ci * 128 : (ci + 1) * 128, 0:TSL]
                )
                nc.sync.dma_start(
                    wqk_sb[ci][:, 0:HQK], wqk_d[ci * 128 : (ci + 1) * 128, 0:HQK]
                )
            nc.sync.dma_start(bqk_sb[:], bqk_d)
            for ci in range(n_ct):
                nc.sync.dma_start(wv_sb[ci][:], wv_d[ci * 128 : (ci + 1) * 128, :])
            nc.sync.dma_start(bv_sb[:], bv_d)
            nc.gpsimd.partition_broadcast(bv_bc[:], bv_sb[:])
            nc.sync.dma_start(mask_sb[:], mask_d)
            for ci in range(n_ct):
                nc.sync.dma_start(
                    wqk_sb[ci][:, HQK : n_qk * 128],
                    wqk_d[ci * 128 : (ci + 1) * 128, HQK : n_qk * 128],
                )
            for ci in range(n_ct):
                nc.sync.dma_start(
                    xT_sb[ci][:, TSL : 2 * TSL],
                    xT_d[ci * 128 : (ci + 1) * 128, TSL : 2 * TSL],
                )
            for p in range(n_pairs):
                nc.sync.dma_start(wo_sb[p][:], wo_d[p * 128 : (p + 1) * 128, :])
            for ii in range(2, n_it):
                for ci in range(n_ct):
                    nc.sync.dma_start(
                        xT_sb[ci][:, ii * TSL : (ii + 1) * TSL],
                        xT_d[ci * 128 : (ci + 1) * 128, ii * TSL : (ii + 1) * TSL],
                    )

            qkT_sb = [
                big_pool.tile([128, T_sz], bf16, tag=f"qkT{ot}", name=f"qkT{ot}")
                for ot in range(n_qk)
            ]
            vext_sb = [
                big_pool.tile([128, VEW], bf16, tag=f"vext{tt}", name=f"vext{tt}")
                for tt in range(n_tt)
            ]
            ctxT_sb = [
                big_pool.tile([128, T_sz], bf16, tag=f"ctxT{p}", name=f"ctxT{p}")
                for p in range(n_pairs)
            ]

            def qk_proj(ot, i):
                ps = mm_ps.tile([128, TSL], f32, tag="mm", name="mm")
                for ci in range(n_ct):
                    nc.tensor.matmul(
                        ps[:],
                        lhsT=wqk_sb[ci][:, ot * 128 : (ot + 1) * 128],
                        rhs=xT_sb[ci][:, i * TSL : (i + 1) * TSL],
                        start=(ci == 0),
                        stop=(ci == n_ct - 1),
                    )
                nc.vector.tensor_scalar_add(
                    qkT_sb[ot][:, i * TSL : (i + 1) * TSL],
                    ps[:],
                    bqk_sb[:, ot : ot + 1],
                )

            def v_proj(tt):
                ps = mm_ps.tile([128, VW], f32, tag="mm", name="mm")
                for ci in range(n_ct):
                    nc.tensor.matmul(
                        ps[:],
                        lhsT=xT_sb[ci][:, tt * TS : (tt + 1) * TS],
                        rhs=wv_sb[ci][:],
                        start=(ci == 0),
                        stop=(ci == n_ct - 1),
                    )
                vx = vext_sb[tt]
                vx3 = vx[:].rearrange("p (h e) -> p h e", e=DK + 1)
                nc.gpsimd.memset(vx3[:, :, DK : DK + 1], 1.0)
                nc.vector.scalar_tensor_tensor(
                    vx3[:, :, 0:DK],
                    ps[:].rearrange("p (h e) -> p h e", e=DK),
                    1.0,
                    bv_bc[:].rearrange("p (h e) -> p h e", e=DK),
                    op0=mybir.AluOpType.mult,
                    op1=mybir.AluOpType.add,
                )

            def out_proj(tt, oh):
                ps = mm_ps.tile([128, OW], f32, tag="mm", name="mm")
                for p in range(n_pairs):
                    nc.tensor.matmul(
                        ps[:],
                        lhsT=ctxT_sb[p][:, tt * TS : (tt + 1) * TS],
                        rhs=wo_sb[p][:, oh * OW : (oh + 1) * OW],
                        start=(p == 0),
                        stop=(p == n_pairs - 1),
                    )
                ob = outsb_pool.tile([128, OW], bf16, tag="outsb", name="outsb")
                nc.vector.tensor_copy(ob[:], ps[:])
                nc.sync.dma_start(
                    out_d[tt * TS : (tt + 1) * TS, oh * OW : (oh + 1) * OW],
                    ob[:],
                )

            # One paced filler queue per iteration. Items tagged must=True
            # (next iter's qk/v projections, deadline = end of this iter)
            # sit in front; out-projection items are all deferred to the
            # LAST iteration, which has by far the largest exp-paced attn
            # phase and no projection work of its own to hide the ScalarE
            # latency behind. A per-pair allowance spreads the fill so late
            # pairs don't run dry, with a tail reserve for the final
            # normalize latency.
            fill_q = deque()
            fill_allow = [0]
            pending_out = []

            def pump_fill(n=1):
                for _ in range(n):
                    if fill_q and fill_allow[0] > 0:
                        fill_q.popleft()[1]()
                        fill_allow[0] -= 1

            def pump_front(n):
                for _ in range(n):
                    if fill_q:
                        fill_q.popleft()[1]()

            mask3 = mask_sb[:].rearrange("p (c w) -> p c w", c=2)

            def score_block(p, i, j):
                """Scores + exp (+ causal mask) for one s-block; returns the
                bf16 attn tile and its first live t column."""
                qt, kt = qkT_sb[2 * p], qkT_sb[2 * p + 1]
                diag = j >= JPI * i
                pi = j - JPI * i if diag else 0
                t0 = pi * TS  # first causally-live t column in this block
                ps = sc_ps.tile([128, 2 * TSL], f32, tag="sc", name="sc")
                nc.tensor.matmul(
                    ps[:, t0:TSL],
                    lhsT=kt[0:64, j * TS : (j + 1) * TS],
                    rhs=qt[0:64, i * TSL + t0 : (i + 1) * TSL],
                    start=True,
                    stop=True,
                    skip_group_check=True,
                )
                nc.tensor.matmul(
                    ps[:, TSL + t0 : 2 * TSL],
                    lhsT=kt[64:128, j * TS : (j + 1) * TS],
                    rhs=qt[64:128, i * TSL + t0 : (i + 1) * TSL],
                    start=True,
                    stop=True,
                    skip_group_check=True,
                )
                a = attn_pool.tile([128, 2 * TSL], bf16, tag="attn", name="attn")
                a3 = a[:].rearrange("p (c w) -> p c w", c=2)
                ps3 = ps[:].rearrange("p (c w) -> p c w", c=2)
                nc.scalar.activation(a3[:, :, t0:TSL], ps3[:, :, t0:TSL], AF.Exp)
                if diag:
                    nc.vector.tensor_mul(
                        a3[:, :, t0 : t0 + TS],
                        a3[:, :, t0 : t0 + TS],
                        mask3[:, :, :],
                    )
                return a, t0

            # score blocks pre-emitted ahead of their pair's body (the last
            # pair's tail overlaps the next pair's first exp latencies)
            heads = {}

            def emit_head(p, i, nh=2):
                heads[(p, i)] = [score_block(p, i, j) for j in range(min(nh, JPI * (i + 1)))]

            def attn_pair(p, i, splice=None):
                nj = JPI * (i + 1)
                pre = heads.pop((p, i), [])
                ctxA = mm_ps.tile([DK + 1, TSL], f32, tag="mm", name="mm")
                ctxB = mm_ps.tile([DK + 1, TSL], f32, tag="mm", name="mm")
                blocks = list(pre)
                for j in range(nj):
                    # keep the score pipeline one block ahead of ctx
                    while len(blocks) <= min(j + 1, nj - 1):
                        blocks.append(score_block(p, i, len(blocks)))
                    if j == nj - 1 and splice is not None:
                        splice()  # next pair's head: exps overlap our tail
                    a, t0 = blocks[j]
                    pump_fill(1)
                    nc.tensor.matmul(
                        ctxA[:, t0:TSL],
                        lhsT=vext_sb[j][:, (2 * p) * (DK + 1) : (2 * p + 1) * (DK + 1)],
                        rhs=a[:, t0:TSL],
                        start=(j == 0),
                        stop=(j == nj - 1),
                    )
                    nc.tensor.matmul(
                        ctxB[:, t0:TSL],
                        lhsT=vext_sb[j][
                            :, (2 * p + 1) * (DK + 1) : (2 * p + 2) * (DK + 1)
                        ],
                        rhs=a[:, TSL + t0 : 2 * TSL],
                        start=(j == 0),
                        stop=(j == nj - 1),
                    )
                isl = slice(i * TSL, (i + 1) * TSL)
                for cps, rows in ((ctxA, slice(0, 64)), (ctxB, slice(64, 128))):
                    # custom-DVE ops misread PSUM on hw: bounce rowsum via SBUF
                    rs = rinv_pool.tile([1, TSL], f32, tag="rsum", name="rsum")
                    nc.vector.tensor_copy(rs[:], cps[DK : DK + 1, :])
                    r = rinv_pool.tile([1, TSL], f32, tag="rinv", name="rinv")
                    nc.vector.reciprocal_approx_fast(r[:], rs[:])
                    rbc = rbc_pool.tile([DK, TSL], f32, tag="rbc", name="rbc")
                    nc.gpsimd.partition_broadcast(rbc[:], r[:])
                    nc.vector.tensor_mul(ctxT_sb[p][rows, isl], cps[0:DK, :], rbc[:])

            # ---- main schedule ----
            for ot in range(n_qk // 2):
                qk_proj(ot, 0)
            for tt in range(JPI):
                v_proj(tt)
            emit_head(0, 0)
            for i in range(n_it):
                if i + 1 < n_it:
                    for ot in range(n_qk):
                        fill_q.append(
                            (True, lambda ot=ot, i=i: qk_proj(ot, i + 1))
                        )
                    for tt in range(JPI * (i + 1), JPI * (i + 2)):
                        fill_q.append((True, lambda tt=tt: v_proj(tt)))
                if i > 0:
                    for tt in range(JPI * (i - 1), JPI * i):
                        for oh in range(n_oh):
                            pending_out.append(
                                lambda tt=tt, oh=oh: out_proj(tt, oh)
                            )
                last_it = i == n_it - 1
                if last_it:
                    for it_ in pending_out:
                        fill_q.append((False, it_))
                    pending_out = []
                for p in range(n_pairs):
                    if i == 0 and p in (1, 2):
                        # qkT for pairs 2/3 must be emitted before their use
                        # (pair p+1's head is spliced into pair p's body)
                        qk_proj(2 * p + 2, 0)
                        qk_proj(2 * p + 3, 0)
                    if i == 0:
                        fill_allow[0] = 0
                        if p > 0:
                            pump_front(4)
                    else:
                        pump_front(2)
                        reserve = 6 if last_it else 0
                        rem = max(0, len(fill_q) - reserve)
                        fill_allow[0] = -(-rem // (n_pairs - p))
                    if p + 1 < n_pairs:
                        nxt = (p + 1, i)
                    elif i + 1 < n_it:
                        nxt = (0, i + 1)
                    else:
                        nxt = None
                    attn_pair(
                        p,
                        i,
                        splice=(lambda nxt=nxt: emit_head(*nxt)) if nxt else None,
                    )
                # any qk/v leftovers must land before the next iteration
                while fill_q and fill_q[0][0]:
                    fill_q.popleft()[1]()
            # tail reserve: fill the last pair's normalize latency and keep
            # the PE clock-gate warm for the final out-projections
            fill_allow[0] = len(fill_q)
            pump_fill(len(fill_q))
            for tt in range(JPI * (n_it - 1), JPI * n_it):
                for oh in range(n_oh):
                    out_proj(tt, oh)

    nc.compile()
    return nc


def make_mask01(ts=TS):
    """[128, 2*ts] bf16 {0,1}: cell (s, t) = 0 iff s > t, two copies."""
    s = np.arange(128)[:, None]
    t = np.arange(ts)[None, :]
    m = np.where(s > t, 0.0, 1.0).astype(np.float32)
    return np.concatenate([m, m], axis=1)


def make_core_inputs(x_b, W_qkv, b_qkv, W_out, heads, C_sz=C, T_sz=T):
    """Build the per-core input map (numpy, host-side)."""
    n_pairs = len(heads) // 2
    n_qk = 2 * n_pairs
    VW = len(heads) * DK
    xT = np.ascontiguousarray(x_b.T).astype(BF16)
    wqk = np.empty((C_sz, n_qk * 128), np.float32)
    bqk = np.empty((128, n_qk), np.float32)
    wv = np.empty((C_sz, VW), np.float32)
    bv = np.empty((1, VW), np.float32)
    wo = np.empty((n_pairs * 128, C_sz), np.float32)
    for p in range(n_pairs):
        hA, hB = heads[2 * p], heads[2 * p + 1]
        # q tile (scaled by 1/sqrt(dk)=1/8), k tile
        for half, h in ((0, hA), (1, hB)):
            r0 = h * 3 * DK
            wqk[:, 2 * p * 128 + half * 64 : 2 * p * 128 + half * 64 + 64] = (
                W_qkv[r0 : r0 + DK].T / math.sqrt(DK)
            )
            bqk[half * 64 : half * 64 + 64, 2 * p] = b_qkv[r0 : r0 + DK] / math.sqrt(DK)
            wqk[:, (2 * p + 1) * 128 + half * 64 : (2 * p + 1) * 128 + half * 64 + 64] = (
                W_qkv[r0 + DK : r0 + 2 * DK].T
            )
            bqk[half * 64 : half * 64 + 64, 2 * p + 1] = b_qkv[r0 + DK : r0 + 2 * DK]
            wo[p * 128 + half * 64 : p * 128 + half * 64 + 64, :] = W_out[
                :, h * DK : (h + 1) * DK
            ].T
    for hh, h in enumerate(heads):
        r0 = h * 3 * DK + 2 * DK
        wv[:, hh * DK : (hh + 1) * DK] = W_qkv[r0 : r0 + DK].T
        bv[0, hh * DK : (hh + 1) * DK] = b_qkv[r0 : r0 + DK]
    return {
        "xT": xT,
        "wqkT": wqk.astype(BF16),
        "wvT": wv.astype(BF16),
        "bqk": bqk.astype(np.float32),
        "bv": bv.astype(BF16),
        "woT": wo.astype(BF16),
        "mask01": make_mask01().astype(BF16),
    }


_NC_CACHE = {}


def kernel(x, W_qkv, b_qkv, W_out, b_out, _trace=False):
    x = np.asarray(x, dtype=np.float32)
    W_qkv = np.asarray(W_qkv, dtype=np.float32)
    b_qkv = np.asarray(b_qkv, dtype=np.float32)
    W_out = np.asarray(W_out, dtype=np.float32)
    b_out = np.asarray(b_out, dtype=np.float32)

    from concourse.bass_utils import run_bass_kernel_spmd

    key = ("full", C, T, 4)
    if key not in _NC_CACHE:
        _NC_CACHE[key] = build_program(C, T, n_pairs=4, num_devices=1)
    nc = _NC_CACHE[key]

    in_maps = []
    for core in range(NCORES):
        b, hg = divmod(core, 2)
        heads = list(range(hg * 8, hg * 8 + 8))
        in_maps.append(make_core_inputs(x[b], W_qkv, b_qkv, W_out, heads))

    res = run_bass_kernel_spmd(nc, in_maps, list(range(NCORES)), trace=_trace)
    kernel._last_results = res

    out = np.broadcast_to(b_out, (B, T, C)).astype(np.float32).copy()
    for core in range(NCORES):
        b = core // 2
        out[b] += np.asarray(res.results[core]["out"], dtype=np.float32)
    return out
